# revision 28
# baseline (speedup 1.0000x reference)
"""Trainium2 Bass kernel for nn_CausalGemAttention.

Reference computation (B=2, T=2048, C=1024, H=16, d=64):
    qkv = x @ w_attn + b_attn ; q,k,v = split(qkv)
    p = sign(sign(p_param)+0.5) * clamp(|p_param|, 1e-4, 1e3)
    vc = clip(|v + 5|, 1e-10); z = p*ln(vc); zmax = max_T(z); v' = exp(z - zmax)
    att = causal_softmax(q k^T / sqrt(d)); mean = att @ v'
    y = exp((zmax + ln(mean)) / p) - 5 ; out = y @ w_proj + b_proj

Sharding: 8 cores = 2 (batch) x 4 (head groups of 4 heads / 256 channels).
Each core computes qkv for its head group (contraction over full C), local
attention, and a partial projection (w_proj rows of its channels); host sums
the 4 partials per batch and adds b_proj.

v2 structure (single pool scope, cross-phase PSUM ring sharing):
  - PSUM map (8 banks exact): tag 'big' 2x[P,2,512]f32 (qkv accum + scores,
    double-buffered so PE never waits on softmax exp), tag 'pv' 1x[65,2,512]
    (paired PV accum), tag 'x' 2x[P,512] (proj/bc/transposes/oc).
  - attention iterates per KEY TILE with both heads of the pair in one
    scores tile; softmax exp is one strided [P,2,512-off] ACT op; only the
    128-wide diagonal block needs a triangular mask ([P,2,128] DVE mult).
  - v'' is pre-scaled by exp(zmax) per channel, and the constant term
    (exp(zmax)*cmid - 5) is folded into a per-core [2,C] vector
    oc = const^T @ w_proj (hi/lo bf16 split) added on host. The pair
    post-chain is: den copy (partition 64->0), fast reciprocal, F32R
    rounding copy, me evacuation, two K=1 broadcast matmuls + multiplies.
  - scores moving operand is raw qT (both heads packed); only the stationary
    k is zero-padded to K=128 (kTp), so no padded q buffer is needed.
  - m=1 qkv/transform units are emitted interleaved into head-pair-0
    attention (between exp and PV) to fill PE gaps; projection and
    post-chains trail one q-block behind.
  - output partials are bf16 (halves the DMA-out + PSUM evacuation cost).
"""

import sys
sys.path.insert(0, "/opt/trn_rl_repo")

import numpy as np
import ml_dtypes

import concourse.bacc as bacc
import concourse.tile as tile
from concourse import mybir
from concourse.bass_utils import run_bass_kernel_spmd

F32 = mybir.dt.float32
F32R = mybir.dt.float32r
BF16 = mybir.dt.bfloat16
AF = mybir.ActivationFunctionType
ALU = mybir.AluOpType
AX = mybir.AxisListType

B, T, C, H, D = 2, 2048, 1024, 16, 64
P = 128
CL = 256            # channels per core (4 heads x 64)
KC = C // P         # 8 contraction chunks for qkv
NQ = T // 512       # 4 query blocks of 512
NK = T // P         # 16 key tiles of 128
SHIFT = 5.0
P_MIN, P_MAX, V_MIN = 1e-4, 1e3, 1e-10
SM_SCALE = 1.0 / 8.0  # 1/sqrt(64)

# cst layout (bf16): [ident128 | tri x2]
CST_W = 128 + 256

_CACHE = {}


def _build(fast_p1, dbg=False):
    nc = bacc.Bacc("TRN2", target_bir_lowering=False, debug=False)

    xt_d = nc.dram_tensor("xt", [P, NQ, KC, 512], BF16,
                          kind="ExternalInput")
    wq_d = nc.dram_tensor("wq", [P, KC, CL], BF16, kind="ExternalInput")
    wk_d = nc.dram_tensor("wk", [P, KC, CL], BF16, kind="ExternalInput")
    wv_d = nc.dram_tensor("wv", [P, KC, CL], BF16, kind="ExternalInput")
    wp_d = nc.dram_tensor("wp", [P, 2, C], BF16, kind="ExternalInput")
    # bps = [bq(2) | bk(2) | bv5(2) | pp(2)] per chunk, fp32
    bps_d = nc.dram_tensor("bps", [P, 8], F32, kind="ExternalInput")
    cst_d = nc.dram_tensor("cst", [P, CST_W], BF16, kind="ExternalInput")
    or_d = nc.dram_tensor("onesr", [1, 64], F32R, kind="ExternalInput")
    out_d = nc.dram_tensor("out_p", [T, C], BF16, kind="ExternalOutput")
    oc_d = nc.dram_tensor("oc", [2, C], F32, kind="ExternalOutput")

    with tile.TileContext(nc) as tc:
        with (
            tc.tile_pool(name="consts", bufs=1) as cp,
            tc.tile_pool(name="main", bufs=1) as mp,
            tc.tile_pool(name="ptp", bufs=6) as ptp,
            tc.tile_pool(name="sm", bufs=2) as smp,
            tc.tile_pool(name="outp", bufs=3) as opp,
            tc.tile_pool(name="psBig", bufs=2, space="PSUM") as psBig,
            tc.tile_pool(name="psPv", bufs=1, space="PSUM") as psPv,
            tc.tile_pool(name="psX", bufs=2, space="PSUM") as psX,
        ):
            # ---------------- constants + inputs ------------------------------
            cst = cp.tile([P, CST_W], BF16)
            onesr = cp.tile([1, 64], F32R)
            bps = cp.tile([P, 8], F32)
            nc.sync.dma_start(bps[:], bps_d[:])
            nc.sync.dma_start(cst[:], cst_d[:])
            nc.sync.dma_start(onesr[:], or_d[:])
            ident = cst[:, 0:128]
            tri2 = cst[:, 128:384].rearrange("p (b c) -> p b c", b=2)
            bq_sb = bps[:, 0:2]
            bk_sb = bps[:, 2:4]
            bv5_sb = bps[:, 4:6]
            pp_sb = bps[:, 6:8]

            wq_sb = mp.tile([P, KC, CL], BF16)
            wk_sb = mp.tile([P, KC, CL], BF16)
            wv_sb = mp.tile([P, KC, CL], BF16)
            wp_sb = mp.tile([P, 2, C], BF16)
            xt_sb = mp.tile([P, NQ, KC, 512], BF16)
            # all inputs are host-prearranged contiguous; x in query-block
            # quarters so the first v matmul group starts ~7us in
            # interleave input pieces across the 3 DMA trigger queues
            # (sync/gpsimd/scalar) so the first query quarter + its weights
            # land concurrently and the feed stays ahead of the qkv matmuls
            nc.sync.dma_start(wv_sb[:], wv_d[:])
            nc.gpsimd.dma_start(wq_sb[:], wq_d[:])
            nc.scalar.dma_start(wk_sb[:], wk_d[:])
            nc.sync.dma_start(xt_sb[:, 0, 0:4], xt_d[:, 0, 0:4])
            nc.gpsimd.dma_start(xt_sb[:, 0, 4:8], xt_d[:, 0, 4:8])
            nc.scalar.dma_start(xt_sb[:, 1, 0:4], xt_d[:, 1, 0:4])
            nc.sync.dma_start(xt_sb[:, 1, 4:8], xt_d[:, 1, 4:8])
            nc.gpsimd.dma_start(xt_sb[:, 2, 0:4], xt_d[:, 2, 0:4])
            nc.scalar.dma_start(xt_sb[:, 2, 4:8], xt_d[:, 2, 4:8])
            nc.sync.dma_start(xt_sb[:, 3, 0:4], xt_d[:, 3, 0:4])
            nc.gpsimd.dma_start(xt_sb[:, 3, 4:8], xt_d[:, 3, 4:8])
            nc.sync.dma_start(wp_sb[:], wp_d[:])

            qT = mp.tile([P, 2, T], BF16)    # q^T: [c%128, c//128, t]
            kT = mp.tile([P, 2, T], BF16)
            # padded per-head k (stationary side only): head h occupies
            # partitions 64*(h%2):64*(h%2)+64, rest zero
            kTp = mp.tile([P, 4, T], BF16)
            vT = mp.tile([P, 2, T], F32)
            vpT = mp.tile([P, 2, T], BF16)
            vnat = mp.tile([P, 4, NK, 65], BF16)  # [tk%128, head, tk//128, d|1]
            yT = mp.tile([P, 2, T], BF16)
            nc.gpsimd.memset(kTp[:], 0.0)
            for h in range(4):
                nc.vector.memset(vnat[:, h, :, 64], 1.0)

            # p = sign(sign(pp)+0.5) * clamp(|pp|, P_MIN, P_MAX)
            sgn = cp.tile([P, 2], F32)
            ab = cp.tile([P, 2], F32)
            p_sb = cp.tile([P, 2], F32)
            # allcp packs [ip | zmaxp | cmid | ezp | ecp5 | eip] x 2 chunks
            allcp = cp.tile([P, 6, 2], F32)
            ip_sb = allcp[:, 0, :]
            zmaxp = allcp[:, 1, :]
            cmid = allcp[:, 2, :]
            ezp = allcp[:, 3, :]
            ecp5 = allcp[:, 4, :]
            eip = allcp[:, 5, :]
            nc.scalar.activation(sgn[:], pp_sb[:], AF.Sign)
            nc.vector.tensor_scalar_add(sgn[:], sgn[:], 0.5)
            nc.scalar.activation(sgn[:], sgn[:], AF.Sign)
            nc.scalar.activation(ab[:], pp_sb[:], AF.Abs)
            nc.vector.tensor_scalar(ab[:], ab[:], float(P_MIN), float(P_MAX),
                                    ALU.max, ALU.min)
            nc.vector.tensor_tensor(p_sb[:], sgn[:], ab[:], ALU.mult)
            nc.vector.reciprocal(ip_sb[:], p_sb[:])

            negzmax = cp.tile([P, 2], F32)
            zmin_sb = cp.tile([P, 2], F32)
            pmax = cp.tile([P, 2, NQ], F32)   # per-quarter -max partials
            pmin = cp.tile([P, 2, NQ], F32)
            # per-head [64,1] base-0 views (general-p path)
            allh = cp.tile([64, 6, 4], F32)
            iph = allh[:, 0, :]
            zmh = allh[:, 1, :]
            cmh = allh[:, 2, :]
            eih = allh[:, 5, :]
            ecb = cp.tile([P, 2, 2], BF16)  # ecp5 (or -SHIFT) hi/lo for oc
            ecl = cp.tile([P, 2], F32)

            # ---------------- emission units ----------------------------------
            def qkv_group(wsb, kind, m, nt):
                ps = psBig.tile([P, 512], F32, tag="big", name="ev")
                for kc in range(KC):
                    nc.tensor.matmul(
                        ps[:],
                        wsb[:, kc, m * P:(m + 1) * P],
                        xt_sb[:, nt, kc, :],
                        start=(kc == 0), stop=(kc == KC - 1),
                    )
                tsl = slice(nt * 512, (nt + 1) * 512)
                if kind == "q":
                    nc.vector.tensor_scalar_add(
                        qT[:, m, tsl], ps[:], bq_sb[:, m:m + 1])
                elif kind == "k":
                    nc.vector.tensor_scalar_add(
                        kT[:, m, tsl], ps[:], bk_sb[:, m:m + 1])
                else:
                    # fast (p=1): vT = |v + b + SHIFT| directly; the power-mean
                    # collapses to a plain max-normalized mean so no ln/exp is
                    # needed (keeps ScalarE on one activation table set).
                    # general: z = p * ln(|v + b + SHIFT|)
                    nc.scalar.activation(
                        vT[:, m, tsl], ps[:], AF.Abs,
                        bias=bv5_sb[:, m:m + 1])
                    if not fast_p1:
                        nc.scalar.activation(vT[:, m, tsl], vT[:, m, tsl],
                                             AF.Ln)
                        nc.vector.tensor_scalar_mul(
                            vT[:, m, tsl], vT[:, m, tsl], p_sb[:, m:m + 1])
                    nc.vector.tensor_reduce(pmax[:, m, nt:nt + 1],
                                            vT[:, m, tsl], AX.X,
                                            op=ALU.max, negate=True)
                    nc.vector.tensor_reduce(pmin[:, m, nt:nt + 1],
                                            vT[:, m, tsl], AX.X, op=ALU.min)

            def v_tail(m):
                nc.vector.tensor_reduce(negzmax[:, m:m + 1], pmax[:, m, :],
                                        AX.X, op=ALU.min)
                nc.vector.tensor_reduce(zmin_sb[:, m:m + 1], pmin[:, m, :],
                                        AX.X, op=ALU.min)
                if fast_p1:
                    # v''*ezp = |v+5| - 0.5*(max+min); ecp5 = 0.5*(max+min)-5
                    # co = 0.5*(max+min) stored in ecp5 slot (pre -SHIFT)
                    nc.vector.scalar_tensor_tensor(
                        ecp5[:, m:m + 1], negzmax[:, m:m + 1], -1.0,
                        zmin_sb[:, m:m + 1], ALU.mult, ALU.add)
                    nc.vector.tensor_scalar_mul(ecp5[:, m:m + 1],
                                                ecp5[:, m:m + 1], 0.5)
                    nc.vector.tensor_scalar_sub(vpT[:, m, :], vT[:, m, :],
                                                ecp5[:, m:m + 1])
                    nc.vector.tensor_scalar_add(ecp5[:, m:m + 1],
                                                ecp5[:, m:m + 1], -SHIFT)
                    return
                # general p: zmax/zmin; cmid = 0.5*(1+exp(zmin-zmax));
                # ezp = exp(zmax); ecp5 = ezp*cmid - 5; v'' = (v'-cmid)*ezp
                nc.vector.scalar_tensor_tensor(
                    zmaxp[:, m:m + 1], negzmax[:, m:m + 1], -1.0,
                    ip_sb[:, m:m + 1], ALU.mult, ALU.mult)
                nc.scalar.activation(cmid[:, m:m + 1], zmin_sb[:, m:m + 1],
                                     AF.Exp, bias=negzmax[:, m:m + 1])
                nc.vector.tensor_scalar(cmid[:, m:m + 1], cmid[:, m:m + 1],
                                        1.0, 0.5, ALU.add, ALU.mult)
                nc.scalar.activation(ezp[:, m:m + 1], negzmax[:, m:m + 1],
                                     AF.Exp, scale=-1.0)
                nc.scalar.activation(eip[:, m:m + 1], negzmax[:, m:m + 1],
                                     AF.Exp)
                nc.vector.scalar_tensor_tensor(
                    ecp5[:, m:m + 1], ezp[:, m:m + 1], 0.0,
                    cmid[:, m:m + 1], ALU.bypass, ALU.mult)
                nc.vector.tensor_scalar_add(ecp5[:, m:m + 1], ecp5[:, m:m + 1],
                                            -SHIFT)
                nc.scalar.activation(vT[:, m, :], vT[:, m, :], AF.Exp,
                                     bias=negzmax[:, m:m + 1])
                nc.vector.tensor_scalar(vpT[:, m, :], vT[:, m, :],
                                        cmid[:, m:m + 1], ezp[:, m:m + 1],
                                        ALU.subtract, ALU.mult)

            def kTp_fill(m):
                for h in (2 * m, 2 * m + 1):
                    base = 64 * (h % 2)
                    nc.gpsimd.dma_start(kTp[base:base + 64, h, :],
                                        kT[base:base + 64, m, :])

            def transpose_unit(m, tp):
                # 4 key tiles -> vnat via one [128,128] PE transpose each
                trp = psX.tile([P, 512], BF16, tag="x", name="trp")
                for j in range(4):
                    kt = 4 * tp + j
                    nc.tensor.transpose(
                        trp[:, j * 128:(j + 1) * 128],
                        vpT[:, m, kt * P:(kt + 1) * P],
                        ident,
                    )
                t3 = trp[:].rearrange("p (a b) -> p a b", a=4)
                nc.vector.tensor_copy(
                    vnat[:, 2 * m, 4 * tp:4 * tp + 4, 0:64], t3[:, :, 0:64])
                nc.vector.tensor_copy(
                    vnat[:, 2 * m + 1, 4 * tp:4 * tp + 4, 0:64],
                    t3[:, :, 64:128])

            def head_consts(m):
                nc.sync.dma_start(allh[:, :, 2 * m], allcp[0:64, :, m])
                nc.sync.dma_start(allh[:, :, 2 * m + 1], allcp[64:128, :, m])

            def m_units(m):
                # main: feed-rate-matched (v,q,k per query quarter) + kTp;
                # tail: v-transform + transposes (deferred into attention
                # fillers so scores can start as soon as kTp is ready)
                units = []
                for nt in range(NQ):
                    units.append(lambda m=m, nt=nt: qkv_group(wv_sb, "v", m, nt))
                    units.append(lambda m=m, nt=nt: qkv_group(wq_sb, "q", m, nt))
                    units.append(lambda m=m, nt=nt: qkv_group(wk_sb, "k", m, nt))
                units.append(lambda m=m: kTp_fill(m))
                tail = [lambda m=m: v_tail(m)]
                for tp in range(4):
                    tail.append(lambda m=m, tp=tp: transpose_unit(m, tp))
                tail.append(lambda m=m: head_consts(m))
                return units, tail

            # ---------------- attention ---------------------------------------
            def post_pair(pv_t, hp, qi):
                # yh = num'' / den per head; const term folded into oc
                dcp = smp.tile([1, 1024], F32, tag="dcp", name="dcp")
                nc.vector.tensor_copy(dcp[:], pv_t[64:65, :, :])
                rdf = smp.tile([1, 1024], F32, tag="rdf", name="rdf")
                nc.vector.reciprocal_approx_fast(rdf[:], dcp[:])
                rd = smp.tile([1, 1024], F32R, tag="rd", name="rd")
                nc.vector.tensor_copy(rd[:], rdf[:])
                me_s = smp.tile([64, 2, 512], F32, tag="me_s",
                                name="me_s")
                nc.vector.tensor_copy(me_s[:], pv_t[0:64, :, :])
                yh = smp.tile([64, 2, 512], BF16, tag="yh", name="yh")
                for i in range(2):
                    h = 2 * hp + i
                    bc = psX.tile([64, 512], F32, tag="x", name="bc",
                                  padded_shape=[P, 512])
                    nc.tensor.matmul(bc[:], onesr[:],
                                     rd[:, i * 512:(i + 1) * 512],
                                     start=True, stop=True)
                    if fast_p1:
                        nc.vector.tensor_tensor(yh[:, i, :], me_s[:, i, :],
                                                bc[:], ALU.mult)
                    else:
                        me = smp.tile([64, 512], F32, tag="me", name="me")
                        nc.vector.tensor_tensor(me[:], me_s[:, i, :], bc[:],
                                                ALU.mult)
                        nc.vector.tensor_scalar(me[:], me[:],
                                                eih[:, h:h + 1],
                                                cmh[:, h:h + 1],
                                                ALU.mult, ALU.add)
                        nc.scalar.activation(me[:], me[:], AF.Ln)
                        nc.scalar.activation(yh[:, i, :], me[:], AF.Exp,
                                             scale=iph[:, h:h + 1],
                                             bias=zmh[:, h:h + 1])
                if dbg and hp == 0 and qi == 0:
                    dpv = cp.tile([65, 1024], F32, name="dpv")
                    nc.vector.tensor_copy(
                        dpv[:], pv_t[:].rearrange("p a b -> p (a b)"))
                    dpv_d = nc.dram_tensor("dpv", [65, 1024], F32,
                                           kind="ExternalOutput")
                    nc.sync.dma_start(dpv_d[:], dpv[:])
                    dyh_d = nc.dram_tensor("dyh", [64, 1024], BF16,
                                           kind="ExternalOutput")
                    nc.sync.dma_start(dyh_d[:],
                                      yh[:].rearrange("p a b -> p (a b)"))
                    drd_d = nc.dram_tensor("drd", [2, 1024], F32,
                                           kind="ExternalOutput")
                    nc.sync.dma_start(drd_d[0:1, :], rdf[:])
                    nc.sync.dma_start(drd_d[1:2, :], dcp[:])
                qi_ = post_qi[0]
                qsl = slice(qi_ * 512, (qi_ + 1) * 512)
                nc.gpsimd.dma_start(yT[0:64, hp, qsl], yh[:, 0, :])
                nc.gpsimd.dma_start(yT[64:128, hp, qsl], yh[:, 1, :])

            post_qi = [0]

            def proj_group(tq):
                po = opp.tile([P, C], BF16, tag="po", name="po")
                for nh in range(2):
                    pj = psX.tile([P, 512], F32, tag="x", name="pj")
                    for c in range(2):
                        nc.tensor.matmul(
                            pj[:],
                            yT[:, c, tq * P:(tq + 1) * P],
                            wp_sb[:, c, nh * 512:(nh + 1) * 512],
                            start=(c == 0), stop=(c == 1),
                        )
                    nc.vector.tensor_copy(po[:, nh * 512:(nh + 1) * 512],
                                          pj[:])
                nc.sync.dma_start(out_d[tq * P:(tq + 1) * P, :], po[:])

            def oc_unit():
                # oc = const^T @ w_proj, const = ecp5 (fast) or -SHIFT
                # (general), hi/lo split across two bf16 stationary columns
                if fast_p1:
                    nc.vector.tensor_copy(ecb[:, :, 0], ecp5[:])
                    nc.vector.tensor_tensor(ecl[:], ecp5[:], ecb[:, :, 0],
                                            ALU.subtract)
                    nc.vector.tensor_copy(ecb[:, :, 1], ecl[:])
                else:
                    nc.vector.memset(ecb[:], 0.0)
                    nc.vector.memset(ecb[:, :, 0], -SHIFT)
                occ = cp.tile([2, C], F32)
                for nh in range(2):
                    ocp = psX.tile([2, 512], F32, tag="x", name="ocp",
                                   padded_shape=[P, 512])
                    for c in range(2):
                        nc.tensor.matmul(
                            ocp[:], ecb[:, c, :],
                            wp_sb[:, c, nh * 512:(nh + 1) * 512],
                            start=(c == 0), stop=(c == 1),
                        )
                    nc.vector.tensor_copy(occ[:, nh * 512:(nh + 1) * 512],
                                          ocp[:])
                nc.sync.dma_start(oc_d[:], occ[:])

            def attention(hp, fillers):
                pending = []
                fill_i = [0]
                n_iters = sum(4 * qi + 4 for qi in range(NQ))
                it = [0]

                def pace():
                    want = (it[0] * len(fillers) + n_iters - 1) // n_iters
                    while fill_i[0] < want:
                        fillers[fill_i[0]]()
                        fill_i[0] += 1

                for qi in range(NQ):
                    nkt = 4 * qi + 4
                    pv = psPv.tile([65, 2, 512], F32, tag="pv", name="pv")
                    prev = None
                    for kt in range(nkt):
                        it[0] += 1
                        off = P * max(kt - 4 * qi, 0)
                        s = psBig.tile([P, 2, 512], F32, tag="big", name="s")
                        for i in range(2):
                            h = 2 * hp + i
                            nc.tensor.matmul(
                                s[:, i, off:512],
                                kTp[:, h, kt * P:(kt + 1) * P],
                                qT[:, hp, qi * 512 + off:(qi + 1) * 512],
                                start=True, stop=True,
                                skip_group_check=True,
                            )
                        pt = ptp.tile([P, 2, 512], BF16, tag="pt", name="pt")
                        nc.scalar.activation(pt[:, :, off:512],
                                             s[:, :, off:512],
                                             AF.Exp, scale=SM_SCALE)
                        if kt >= 4 * qi:   # diagonal: mask the 128-block
                            nc.vector.tensor_mul(pt[:, :, off:off + P],
                                                 pt[:, :, off:off + P],
                                                 tri2[:])
                        if dbg and hp == 0 and qi == 0 and kt == 0:
                            dpt_d = nc.dram_tensor("dpt", [P, 1024], BF16,
                                                   kind="ExternalOutput")
                            nc.sync.dma_start(
                                dpt_d[:], pt[:].rearrange("p a b -> p (a b)"))
                            ds_t = cp.tile([P, 1024], F32, name="ds_t")
                            nc.vector.tensor_copy(
                                ds_t[:], s[:].rearrange("p a b -> p (a b)"))
                            ds_d = nc.dram_tensor("ds", [P, 1024], F32,
                                                  kind="ExternalOutput")
                            nc.sync.dma_start(ds_d[:], ds_t[:])
                        pace()
                        if kt == 1:
                            for fn in pending:
                                fn()
                            pending.clear()
                            if hp == 1 and qi > 0:
                                for tq in range(4 * (qi - 1), 4 * qi):
                                    proj_group(tq)
                        if prev is not None:
                            pkt, ppt = prev
                            o_ = P * max(pkt - 4 * qi, 0)
                            for i in range(2):
                                h = 2 * hp + i
                                nc.tensor.matmul(
                                    pv[:, i, o_:512],
                                    vnat[:, h, pkt, :],
                                    ppt[:, i, o_:512],
                                    start=(pkt == 0), stop=(pkt == nkt - 1),
                                    skip_group_check=True,
                                )
                        prev = (kt, pt)
                    pkt, ppt = prev
                    o_ = P * max(pkt - 4 * qi, 0)
                    for i in range(2):
                        h = 2 * hp + i
                        nc.tensor.matmul(
                            pv[:, i, o_:512],
                            vnat[:, h, pkt, :],
                            ppt[:, i, o_:512],
                            start=(pkt == 0), stop=(pkt == nkt - 1),
                            skip_group_check=True,
                        )

                    def mk(pv_t, hp_, qi_):
                        def fn():
                            post_qi[0] = qi_
                            post_pair(pv_t, hp_, qi_)
                        return fn
                    pending.append(mk(pv, hp, qi))
                while fill_i[0] < len(fillers):
                    fillers[fill_i[0]]()
                    fill_i[0] += 1
                return pending

            # ---------------- schedule ----------------------------------------
            m0_main, m0_tail = m_units(0)
            m1_main, m1_tail = m_units(1)
            for u in m0_main + m0_tail:
                u()
            pending = attention(0, m1_main + m1_tail)
            for fn in pending:
                fn()
            pending = attention(1, [oc_unit])
            for fn in pending:
                fn()
            for tq in range(12, 16):
                proj_group(tq)

            if dbg:
                for nm, t in [("dq", qT), ("dk", kT), ("dkp", kTp),
                              ("dvp", vpT), ("dy", yT)]:
                    dd = nc.dram_tensor(nm, list(t.shape), BF16,
                                        kind="ExternalOutput")
                    nc.sync.dma_start(dd[:], t[:])
                dvn = nc.dram_tensor("dvn", list(vnat.shape), BF16,
                                     kind="ExternalOutput")
                nc.sync.dma_start(dvn[:], vnat[:])

    nc.finalize()
    return nc


def _host_inputs(x, w_attn, b_attn, w_proj, p_param):
    """Build the 8 per-core input dicts."""
    bf16 = ml_dtypes.bfloat16
    ident = np.eye(P, dtype=np.float32)
    xx = np.arange(P, dtype=np.int64)[:, None]
    yy = np.arange(P, dtype=np.int64)[None, :]
    tri = (yy - xx >= 0).astype(np.float32)
    cst = np.concatenate([ident, tri, tri], axis=1).astype(bf16)
    onesr = np.ones((1, 64), dtype=np.float32)

    def warr(w):  # [C, n] -> [P, KC, n] contiguous
        n = w.shape[1]
        return np.ascontiguousarray(
            w.reshape(KC, P, n).transpose(1, 0, 2)).astype(bf16)

    # x^T quarters: [P, NQ, KC, 512]
    xts = []
    for b in range(B):
        xt = x[b].T.reshape(KC, P, NQ, 512)
        xts.append(np.ascontiguousarray(xt.transpose(1, 2, 0, 3)).astype(bf16))
    in_maps = []
    for core in range(8):
        b, hg = divmod(core, 4)
        cs = slice(hg * CL, (hg + 1) * CL)
        csC = slice(C + hg * CL, C + (hg + 1) * CL)
        cs2C = slice(2 * C + hg * CL, 2 * C + (hg + 1) * CL)
        in_maps.append({
            "xt": xts[b],
            "wq": warr(w_attn[:, cs]),
            "wk": warr(w_attn[:, csC]),
            "wv": warr(w_attn[:, cs2C]),
            "wp": np.ascontiguousarray(
                w_proj[cs, :].reshape(2, P, C).transpose(1, 0, 2)).astype(bf16),
            "bps": np.ascontiguousarray(np.concatenate([
                b_attn[cs].reshape(2, P).T,
                b_attn[csC].reshape(2, P).T,
                (b_attn[cs2C] + SHIFT).reshape(2, P).T,
                p_param[cs].reshape(2, P).T,
            ], axis=1).astype(np.float32)),
            "cst": cst,
            "onesr": onesr,
        })
    return in_maps


def kernel(x, w_attn, b_attn, w_proj, b_proj, p_param, _trace=False):
    x = np.asarray(x, dtype=np.float32)
    w_attn = np.asarray(w_attn, dtype=np.float32)
    b_attn = np.asarray(b_attn, dtype=np.float32)
    w_proj = np.asarray(w_proj, dtype=np.float32)
    b_proj = np.asarray(b_proj, dtype=np.float32)
    p_param = np.asarray(p_param, dtype=np.float32)

    # p == 1 admits a cheaper final transform (no per-tile ln/exp)
    p_eff = np.sign(np.sign(p_param) + 0.5) * np.clip(np.abs(p_param),
                                                      P_MIN, P_MAX)
    fast_p1 = bool(np.all(p_eff == 1.0))

    key = ("nc", fast_p1)
    if key not in _CACHE:
        _CACHE[key] = _build(fast_p1)
    nc = _CACHE[key]

    in_maps = _host_inputs(x, w_attn, b_attn, w_proj, p_param)
    res = run_bass_kernel_spmd(nc, in_maps, core_ids=list(range(8)),
                               trace=_trace)
    _CACHE["last_result"] = res

    out = np.zeros((B, T, C), dtype=np.float32)
    for core in range(8):
        b = core // 4
        out[b] += res.results[core]["out_p"].astype(np.float32)
        out[b] += res.results[core]["oc"].astype(np.float32).sum(0)
    out += b_proj[None, None, :]
    return out


if __name__ == "__main__":
    rng = np.random.default_rng(0)
    ins = {
        "x": rng.standard_normal((B, T, C), dtype=np.float32),
        "w_attn": (rng.standard_normal((C, 3 * C), dtype=np.float32) * 0.02),
        "b_attn": np.zeros(3 * C, np.float32),
        "w_proj": (rng.standard_normal((C, C), dtype=np.float32) * 0.02),
        "b_proj": np.zeros(C, np.float32),
        "p_param": np.ones(C, np.float32),
    }
    out = kernel(**ins)
    print("ran, out shape", out.shape, "finite:", np.isfinite(out).all())


# revision 29
# speedup vs baseline: 1.0482x; 1.0482x over previous
"""Trainium2 Bass kernel for nn_CausalGemAttention.

Reference computation (B=2, T=2048, C=1024, H=16, d=64):
    qkv = x @ w_attn + b_attn ; q,k,v = split(qkv)
    p = sign(sign(p_param)+0.5) * clamp(|p_param|, 1e-4, 1e3)
    vc = clip(|v + 5|, 1e-10); z = p*ln(vc); zmax = max_T(z); v' = exp(z - zmax)
    att = causal_softmax(q k^T / sqrt(d)); mean = att @ v'
    y = exp((zmax + ln(mean)) / p) - 5 ; out = y @ w_proj + b_proj

Sharding: 8 cores = 2 (batch) x 4 (head groups of 4 heads / 256 channels).
Each core computes qkv for its head group (contraction over full C), local
attention, and a partial projection (w_proj rows of its channels); host sums
the 4 partials per batch and adds b_proj.

v2 structure (single pool scope, cross-phase PSUM ring sharing):
  - PSUM map (8 banks exact): tag 'big' 2x[P,2,512]f32 (qkv accum + scores,
    double-buffered so PE never waits on softmax exp), tag 'pv' 1x[65,2,512]
    (paired PV accum), tag 'x' 2x[P,512] (proj/bc/transposes/oc).
  - attention iterates per KEY TILE with both heads of the pair in one
    scores tile; softmax exp is one strided [P,2,512-off] ACT op; only the
    128-wide diagonal block needs a triangular mask ([P,2,128] DVE mult).
  - v'' is pre-scaled by exp(zmax) per channel, and the constant term
    (exp(zmax)*cmid - 5) is folded into a per-core [2,C] vector
    oc = const^T @ w_proj (hi/lo bf16 split) added on host. The pair
    post-chain is: den copy (partition 64->0), fast reciprocal, F32R
    rounding copy, me evacuation, two K=1 broadcast matmuls + multiplies.
  - scores moving operand is raw qT (both heads packed); only the stationary
    k is zero-padded to K=128 (kTp), so no padded q buffer is needed.
  - m=1 qkv/transform units are emitted interleaved into head-pair-0
    attention (between exp and PV) to fill PE gaps; projection and
    post-chains trail one q-block behind.
  - output partials are bf16 (halves the DMA-out + PSUM evacuation cost).
"""

import sys
sys.path.insert(0, "/opt/trn_rl_repo")

import numpy as np
import ml_dtypes

import concourse.bacc as bacc
import concourse.tile as tile
from concourse import mybir
from concourse.bass_utils import run_bass_kernel_spmd

F32 = mybir.dt.float32
F32R = mybir.dt.float32r
BF16 = mybir.dt.bfloat16
AF = mybir.ActivationFunctionType
ALU = mybir.AluOpType
AX = mybir.AxisListType

B, T, C, H, D = 2, 2048, 1024, 16, 64
P = 128
CL = 256            # channels per core (4 heads x 64)
KC = C // P         # 8 contraction chunks for qkv
NQ = T // 512       # 4 query blocks of 512
NK = T // P         # 16 key tiles of 128
SHIFT = 5.0
P_MIN, P_MAX, V_MIN = 1e-4, 1e3, 1e-10
SM_SCALE = 1.0 / 8.0  # 1/sqrt(64)

# cst layout (bf16): [ident128 | tri x2]
CST_W = 128 + 256

_CACHE = {}


def _build(fast_p1, dbg=False):
    nc = bacc.Bacc("TRN2", target_bir_lowering=False, debug=False)

    xt_d = nc.dram_tensor("xt", [P, NQ, KC, 512], BF16,
                          kind="ExternalInput")
    wq_d = nc.dram_tensor("wq", [P, KC, CL], BF16, kind="ExternalInput")
    wk_d = nc.dram_tensor("wk", [P, KC, CL], BF16, kind="ExternalInput")
    wv_d = nc.dram_tensor("wv", [P, KC, CL], BF16, kind="ExternalInput")
    wp_d = nc.dram_tensor("wp", [P, 2, C], BF16, kind="ExternalInput")
    # bps = [bq(2) | bk(2) | bv5(2) | pp(2)] per chunk, fp32
    bps_d = nc.dram_tensor("bps", [P, 8], F32, kind="ExternalInput")
    cst_d = nc.dram_tensor("cst", [P, CST_W], BF16, kind="ExternalInput")
    or_d = nc.dram_tensor("onesr", [1, 64], F32R, kind="ExternalInput")
    out_d = nc.dram_tensor("out_p", [T, C], BF16, kind="ExternalOutput")
    oc_d = nc.dram_tensor("oc", [2, C], F32, kind="ExternalOutput")

    with tile.TileContext(nc) as tc:
        with (
            tc.tile_pool(name="consts", bufs=1) as cp,
            tc.tile_pool(name="main", bufs=1) as mp,
            tc.tile_pool(name="ptp", bufs=6) as ptp,
            tc.tile_pool(name="sm", bufs=2) as smp,
            tc.tile_pool(name="outp", bufs=3) as opp,
            tc.tile_pool(name="psBig", bufs=2, space="PSUM") as psBig,
            tc.tile_pool(name="psPv", bufs=1, space="PSUM") as psPv,
            tc.tile_pool(name="psX", bufs=2, space="PSUM") as psX,
        ):
            # ---------------- constants + inputs ------------------------------
            cst = cp.tile([P, CST_W], BF16)
            onesr = cp.tile([1, 64], F32R)
            bps = cp.tile([P, 8], F32)
            nc.sync.dma_start(bps[:], bps_d[:])
            nc.sync.dma_start(cst[:], cst_d[:])
            nc.sync.dma_start(onesr[:], or_d[:])
            ident = cst[:, 0:128]
            tri2 = cst[:, 128:384].rearrange("p (b c) -> p b c", b=2)
            bq_sb = bps[:, 0:2]
            bk_sb = bps[:, 2:4]
            bv5_sb = bps[:, 4:6]
            pp_sb = bps[:, 6:8]

            wq_sb = mp.tile([P, KC, CL], BF16)
            wk_sb = mp.tile([P, KC, CL], BF16)
            wv_sb = mp.tile([P, KC, CL], BF16)
            wp_sb = mp.tile([P, 2, C], BF16)
            xt_sb = mp.tile([P, NQ, KC, 512], BF16)
            # all inputs are host-prearranged contiguous; x in query-block
            # quarters so the first v matmul group starts ~7us in
            # wave 1: only what the first query quarter needs, one piece
            # per trigger queue; later quarters are triggered from the
            # SCALAR queue between the per-quarter |v| ops, so they enter
            # the (round-robin) DMA queue only as the previous quarter is
            # being consumed and don't dilute wave-1 bandwidth.
            nc.sync.dma_start(wv_sb[:], wv_d[:])
            nc.gpsimd.dma_start(xt_sb[:, 0, 0:4], xt_d[:, 0, 0:4])
            nc.scalar.dma_start(xt_sb[:, 0, 4:8], xt_d[:, 0, 4:8])
            nc.gpsimd.dma_start(wq_sb[:], wq_d[:])
            nc.scalar.dma_start(wk_sb[:], wk_d[:])

            qT = mp.tile([P, 2, T], BF16)    # q^T: [c%128, c//128, t]
            kT = mp.tile([P, 2, T], BF16)
            # padded per-head k (stationary side only): head h occupies
            # partitions 64*(h%2):64*(h%2)+64, rest zero
            kTp = mp.tile([P, 4, T], BF16)
            vT = mp.tile([P, 2, T], F32)
            vpT = mp.tile([P, 2, T], BF16)
            vnat = mp.tile([P, 4, NK, 65], BF16)  # [tk%128, head, tk//128, d|1]
            yT = mp.tile([P, 2, T], BF16)
            nc.gpsimd.memset(kTp[:], 0.0)
            for h in range(4):
                nc.vector.memset(vnat[:, h, :, 64], 1.0)

            # p = sign(sign(pp)+0.5) * clamp(|pp|, P_MIN, P_MAX)
            sgn = cp.tile([P, 2], F32)
            ab = cp.tile([P, 2], F32)
            p_sb = cp.tile([P, 2], F32)
            # allcp packs [ip | zmaxp | cmid | ezp | ecp5 | eip] x 2 chunks
            allcp = cp.tile([P, 6, 2], F32)
            ip_sb = allcp[:, 0, :]
            zmaxp = allcp[:, 1, :]
            cmid = allcp[:, 2, :]
            ezp = allcp[:, 3, :]
            ecp5 = allcp[:, 4, :]
            eip = allcp[:, 5, :]
            nc.scalar.activation(sgn[:], pp_sb[:], AF.Sign)
            nc.vector.tensor_scalar_add(sgn[:], sgn[:], 0.5)
            nc.scalar.activation(sgn[:], sgn[:], AF.Sign)
            nc.scalar.activation(ab[:], pp_sb[:], AF.Abs)
            nc.vector.tensor_scalar(ab[:], ab[:], float(P_MIN), float(P_MAX),
                                    ALU.max, ALU.min)
            nc.vector.tensor_tensor(p_sb[:], sgn[:], ab[:], ALU.mult)
            nc.vector.reciprocal(ip_sb[:], p_sb[:])

            negzmax = cp.tile([P, 2], F32)
            zmin_sb = cp.tile([P, 2], F32)
            pmax = cp.tile([P, 2, NQ], F32)   # per-quarter -max partials
            pmin = cp.tile([P, 2, NQ], F32)
            # per-head [64,1] base-0 views (general-p path)
            allh = cp.tile([64, 6, 4], F32)
            iph = allh[:, 0, :]
            zmh = allh[:, 1, :]
            cmh = allh[:, 2, :]
            eih = allh[:, 5, :]
            ecb = cp.tile([P, 2, 2], BF16)  # ecp5 (or -SHIFT) hi/lo for oc
            ecl = cp.tile([P, 2], F32)

            # ---------------- emission units ----------------------------------
            def qkv_group(wsb, kind, m, nt):
                ps = psBig.tile([P, 512], F32, tag="big", name="ev")
                for kc in range(KC):
                    nc.tensor.matmul(
                        ps[:],
                        wsb[:, kc, m * P:(m + 1) * P],
                        xt_sb[:, nt, kc, :],
                        start=(kc == 0), stop=(kc == KC - 1),
                    )
                tsl = slice(nt * 512, (nt + 1) * 512)
                if kind == "q":
                    nc.vector.tensor_scalar_add(
                        qT[:, m, tsl], ps[:], bq_sb[:, m:m + 1])
                elif kind == "k":
                    nc.vector.tensor_scalar_add(
                        kT[:, m, tsl], ps[:], bk_sb[:, m:m + 1])
                else:
                    # fast (p=1): vT = |v + b + SHIFT| directly; the power-mean
                    # collapses to a plain max-normalized mean so no ln/exp is
                    # needed (keeps ScalarE on one activation table set).
                    # general: z = p * ln(|v + b + SHIFT|)
                    nc.scalar.activation(
                        vT[:, m, tsl], ps[:], AF.Abs,
                        bias=bv5_sb[:, m:m + 1])
                    if m == 0 and nt < NQ - 1:
                        nc.scalar.dma_start(xt_sb[:, nt + 1, 0:4],
                                            xt_d[:, nt + 1, 0:4])
                        nc.scalar.dma_start(xt_sb[:, nt + 1, 4:8],
                                            xt_d[:, nt + 1, 4:8])
                    elif m == 0:
                        nc.scalar.dma_start(wp_sb[:], wp_d[:])
                    if not fast_p1:
                        nc.scalar.activation(vT[:, m, tsl], vT[:, m, tsl],
                                             AF.Ln)
                        nc.vector.tensor_scalar_mul(
                            vT[:, m, tsl], vT[:, m, tsl], p_sb[:, m:m + 1])
                    nc.vector.tensor_reduce(pmax[:, m, nt:nt + 1],
                                            vT[:, m, tsl], AX.X,
                                            op=ALU.max, negate=True)
                    nc.vector.tensor_reduce(pmin[:, m, nt:nt + 1],
                                            vT[:, m, tsl], AX.X, op=ALU.min)

            def v_tail(m):
                nc.vector.tensor_reduce(negzmax[:, m:m + 1], pmax[:, m, :],
                                        AX.X, op=ALU.min)
                nc.vector.tensor_reduce(zmin_sb[:, m:m + 1], pmin[:, m, :],
                                        AX.X, op=ALU.min)
                if fast_p1:
                    # v''*ezp = |v+5| - 0.5*(max+min); ecp5 = 0.5*(max+min)-5
                    # co = 0.5*(max+min) stored in ecp5 slot (pre -SHIFT)
                    nc.vector.scalar_tensor_tensor(
                        ecp5[:, m:m + 1], negzmax[:, m:m + 1], -1.0,
                        zmin_sb[:, m:m + 1], ALU.mult, ALU.add)
                    nc.vector.tensor_scalar_mul(ecp5[:, m:m + 1],
                                                ecp5[:, m:m + 1], 0.5)
                    nc.vector.tensor_scalar_sub(vpT[:, m, :], vT[:, m, :],
                                                ecp5[:, m:m + 1])
                    nc.vector.tensor_scalar_add(ecp5[:, m:m + 1],
                                                ecp5[:, m:m + 1], -SHIFT)
                    return
                # general p: zmax/zmin; cmid = 0.5*(1+exp(zmin-zmax));
                # ezp = exp(zmax); ecp5 = ezp*cmid - 5; v'' = (v'-cmid)*ezp
                nc.vector.scalar_tensor_tensor(
                    zmaxp[:, m:m + 1], negzmax[:, m:m + 1], -1.0,
                    ip_sb[:, m:m + 1], ALU.mult, ALU.mult)
                nc.scalar.activation(cmid[:, m:m + 1], zmin_sb[:, m:m + 1],
                                     AF.Exp, bias=negzmax[:, m:m + 1])
                nc.vector.tensor_scalar(cmid[:, m:m + 1], cmid[:, m:m + 1],
                                        1.0, 0.5, ALU.add, ALU.mult)
                nc.scalar.activation(ezp[:, m:m + 1], negzmax[:, m:m + 1],
                                     AF.Exp, scale=-1.0)
                nc.scalar.activation(eip[:, m:m + 1], negzmax[:, m:m + 1],
                                     AF.Exp)
                nc.vector.scalar_tensor_tensor(
                    ecp5[:, m:m + 1], ezp[:, m:m + 1], 0.0,
                    cmid[:, m:m + 1], ALU.bypass, ALU.mult)
                nc.vector.tensor_scalar_add(ecp5[:, m:m + 1], ecp5[:, m:m + 1],
                                            -SHIFT)
                nc.scalar.activation(vT[:, m, :], vT[:, m, :], AF.Exp,
                                     bias=negzmax[:, m:m + 1])
                nc.vector.tensor_scalar(vpT[:, m, :], vT[:, m, :],
                                        cmid[:, m:m + 1], ezp[:, m:m + 1],
                                        ALU.subtract, ALU.mult)

            def kTp_fill(m):
                for h in (2 * m, 2 * m + 1):
                    base = 64 * (h % 2)
                    nc.gpsimd.dma_start(kTp[base:base + 64, h, :],
                                        kT[base:base + 64, m, :])

            def transpose_unit(m, tp):
                # 4 key tiles -> vnat via one [128,128] PE transpose each
                trp = psX.tile([P, 512], BF16, tag="x", name="trp")
                for j in range(4):
                    kt = 4 * tp + j
                    nc.tensor.transpose(
                        trp[:, j * 128:(j + 1) * 128],
                        vpT[:, m, kt * P:(kt + 1) * P],
                        ident,
                    )
                t3 = trp[:].rearrange("p (a b) -> p a b", a=4)
                nc.vector.tensor_copy(
                    vnat[:, 2 * m, 4 * tp:4 * tp + 4, 0:64], t3[:, :, 0:64])
                nc.vector.tensor_copy(
                    vnat[:, 2 * m + 1, 4 * tp:4 * tp + 4, 0:64],
                    t3[:, :, 64:128])

            def head_consts(m):
                nc.sync.dma_start(allh[:, :, 2 * m], allcp[0:64, :, m])
                nc.sync.dma_start(allh[:, :, 2 * m + 1], allcp[64:128, :, m])

            def m_units(m):
                # main: feed-rate-matched (v,q,k per query quarter) + kTp;
                # tail: v-transform + transposes (deferred into attention
                # fillers so scores can start as soon as kTp is ready)
                units = []
                for nt in range(NQ):
                    units.append(lambda m=m, nt=nt: qkv_group(wv_sb, "v", m, nt))
                    units.append(lambda m=m, nt=nt: qkv_group(wq_sb, "q", m, nt))
                    units.append(lambda m=m, nt=nt: qkv_group(wk_sb, "k", m, nt))
                units.append(lambda m=m: kTp_fill(m))
                tail = [lambda m=m: v_tail(m)]
                for tp in range(4):
                    tail.append(lambda m=m, tp=tp: transpose_unit(m, tp))
                tail.append(lambda m=m: head_consts(m))
                return units, tail

            # ---------------- attention ---------------------------------------
            def post_pair(pv_t, hp, qi):
                # yh = num'' / den per head; const term folded into oc
                dcp = smp.tile([1, 1024], F32, tag="dcp", name="dcp")
                nc.vector.tensor_copy(dcp[:], pv_t[64:65, :, :])
                rdf = smp.tile([1, 1024], F32, tag="rdf", name="rdf")
                nc.vector.reciprocal_approx_fast(rdf[:], dcp[:])
                rd = smp.tile([1, 1024], F32R, tag="rd", name="rd")
                nc.vector.tensor_copy(rd[:], rdf[:])
                me_s = smp.tile([64, 2, 512], F32, tag="me_s",
                                name="me_s")
                nc.vector.tensor_copy(me_s[:], pv_t[0:64, :, :])
                yh = smp.tile([64, 2, 512], BF16, tag="yh", name="yh")
                for i in range(2):
                    h = 2 * hp + i
                    bc = psX.tile([64, 512], F32, tag="x", name="bc",
                                  padded_shape=[P, 512])
                    nc.tensor.matmul(bc[:], onesr[:],
                                     rd[:, i * 512:(i + 1) * 512],
                                     start=True, stop=True)
                    if fast_p1:
                        nc.vector.tensor_tensor(yh[:, i, :], me_s[:, i, :],
                                                bc[:], ALU.mult)
                    else:
                        me = smp.tile([64, 512], F32, tag="me", name="me")
                        nc.vector.tensor_tensor(me[:], me_s[:, i, :], bc[:],
                                                ALU.mult)
                        nc.vector.tensor_scalar(me[:], me[:],
                                                eih[:, h:h + 1],
                                                cmh[:, h:h + 1],
                                                ALU.mult, ALU.add)
                        nc.scalar.activation(me[:], me[:], AF.Ln)
                        nc.scalar.activation(yh[:, i, :], me[:], AF.Exp,
                                             scale=iph[:, h:h + 1],
                                             bias=zmh[:, h:h + 1])
                if dbg and hp == 0 and qi == 0:
                    dpv = cp.tile([65, 1024], F32, name="dpv")
                    nc.vector.tensor_copy(
                        dpv[:], pv_t[:].rearrange("p a b -> p (a b)"))
                    dpv_d = nc.dram_tensor("dpv", [65, 1024], F32,
                                           kind="ExternalOutput")
                    nc.sync.dma_start(dpv_d[:], dpv[:])
                    dyh_d = nc.dram_tensor("dyh", [64, 1024], BF16,
                                           kind="ExternalOutput")
                    nc.sync.dma_start(dyh_d[:],
                                      yh[:].rearrange("p a b -> p (a b)"))
                    drd_d = nc.dram_tensor("drd", [2, 1024], F32,
                                           kind="ExternalOutput")
                    nc.sync.dma_start(drd_d[0:1, :], rdf[:])
                    nc.sync.dma_start(drd_d[1:2, :], dcp[:])
                qi_ = post_qi[0]
                qsl = slice(qi_ * 512, (qi_ + 1) * 512)
                nc.gpsimd.dma_start(yT[0:64, hp, qsl], yh[:, 0, :])
                nc.gpsimd.dma_start(yT[64:128, hp, qsl], yh[:, 1, :])

            post_qi = [0]

            def proj_group(tq):
                po = opp.tile([P, C], BF16, tag="po", name="po")
                for nh in range(2):
                    pj = psX.tile([P, 512], F32, tag="x", name="pj")
                    for c in range(2):
                        nc.tensor.matmul(
                            pj[:],
                            yT[:, c, tq * P:(tq + 1) * P],
                            wp_sb[:, c, nh * 512:(nh + 1) * 512],
                            start=(c == 0), stop=(c == 1),
                        )
                    nc.vector.tensor_copy(po[:, nh * 512:(nh + 1) * 512],
                                          pj[:])
                    nc.sync.dma_start(
                        out_d[tq * P:(tq + 1) * P, nh * 512:(nh + 1) * 512],
                        po[:, nh * 512:(nh + 1) * 512])

            def oc_unit():
                # oc = const^T @ w_proj, const = ecp5 (fast) or -SHIFT
                # (general), hi/lo split across two bf16 stationary columns
                if fast_p1:
                    nc.vector.tensor_copy(ecb[:, :, 0], ecp5[:])
                    nc.vector.tensor_tensor(ecl[:], ecp5[:], ecb[:, :, 0],
                                            ALU.subtract)
                    nc.vector.tensor_copy(ecb[:, :, 1], ecl[:])
                else:
                    nc.vector.memset(ecb[:], 0.0)
                    nc.vector.memset(ecb[:, :, 0], -SHIFT)
                occ = cp.tile([2, C], F32)
                for nh in range(2):
                    ocp = psX.tile([2, 512], F32, tag="x", name="ocp",
                                   padded_shape=[P, 512])
                    for c in range(2):
                        nc.tensor.matmul(
                            ocp[:], ecb[:, c, :],
                            wp_sb[:, c, nh * 512:(nh + 1) * 512],
                            start=(c == 0), stop=(c == 1),
                        )
                    nc.vector.tensor_copy(occ[:, nh * 512:(nh + 1) * 512],
                                          ocp[:])
                nc.sync.dma_start(oc_d[:], occ[:])

            def attention(hp, fillers):
                pending = []
                fill_i = [0]
                n_iters = sum(4 * qi + 4 for qi in range(NQ))
                it = [0]

                def pace():
                    want = min(len(fillers),
                               3 + it[0] * max(len(fillers) - 3, 0) // n_iters)
                    while fill_i[0] < want:
                        fillers[fill_i[0]]()
                        fill_i[0] += 1

                for qi in range(NQ):
                    nkt = 4 * qi + 4
                    pv = psPv.tile([65, 2, 512], F32, tag="pv", name="pv")
                    prev = None
                    for kt in range(nkt):
                        it[0] += 1
                        off = P * max(kt - 4 * qi, 0)
                        s = psBig.tile([P, 2, 512], F32, tag="big", name="s")
                        for i in range(2):
                            h = 2 * hp + i
                            nc.tensor.matmul(
                                s[:, i, off:512],
                                kTp[:, h, kt * P:(kt + 1) * P],
                                qT[:, hp, qi * 512 + off:(qi + 1) * 512],
                                start=True, stop=True,
                                skip_group_check=True,
                            )
                        pt = ptp.tile([P, 2, 512], BF16, tag="pt", name="pt")
                        nc.scalar.activation(pt[:, :, off:512],
                                             s[:, :, off:512],
                                             AF.Exp, scale=SM_SCALE)
                        if kt >= 4 * qi:   # diagonal: mask the 128-block
                            nc.vector.tensor_mul(pt[:, :, off:off + P],
                                                 pt[:, :, off:off + P],
                                                 tri2[:])
                        if dbg and hp == 0 and qi == 0 and kt == 0:
                            dpt_d = nc.dram_tensor("dpt", [P, 1024], BF16,
                                                   kind="ExternalOutput")
                            nc.sync.dma_start(
                                dpt_d[:], pt[:].rearrange("p a b -> p (a b)"))
                            ds_t = cp.tile([P, 1024], F32, name="ds_t")
                            nc.vector.tensor_copy(
                                ds_t[:], s[:].rearrange("p a b -> p (a b)"))
                            ds_d = nc.dram_tensor("ds", [P, 1024], F32,
                                                  kind="ExternalOutput")
                            nc.sync.dma_start(ds_d[:], ds_t[:])
                        pace()
                        if kt == 1:
                            for fn in pending:
                                fn()
                            pending.clear()
                            if hp == 1 and qi > 0:
                                for tq in range(4 * (qi - 1), 4 * qi):
                                    proj_group(tq)
                        if prev is not None:
                            pkt, ppt = prev
                            o_ = P * max(pkt - 4 * qi, 0)
                            for i in range(2):
                                h = 2 * hp + i
                                nc.tensor.matmul(
                                    pv[:, i, o_:512],
                                    vnat[:, h, pkt, :],
                                    ppt[:, i, o_:512],
                                    start=(pkt == 0), stop=(pkt == nkt - 1),
                                    skip_group_check=True,
                                )
                        prev = (kt, pt)
                    pkt, ppt = prev
                    o_ = P * max(pkt - 4 * qi, 0)
                    for i in range(2):
                        h = 2 * hp + i
                        nc.tensor.matmul(
                            pv[:, i, o_:512],
                            vnat[:, h, pkt, :],
                            ppt[:, i, o_:512],
                            start=(pkt == 0), stop=(pkt == nkt - 1),
                            skip_group_check=True,
                        )

                    def mk(pv_t, hp_, qi_):
                        def fn():
                            post_qi[0] = qi_
                            post_pair(pv_t, hp_, qi_)
                        return fn
                    pending.append(mk(pv, hp, qi))
                while fill_i[0] < len(fillers):
                    fillers[fill_i[0]]()
                    fill_i[0] += 1
                return pending

            # ---------------- schedule ----------------------------------------
            m0_main, m0_tail = m_units(0)
            m1_main, m1_tail = m_units(1)
            for u in m0_main:
                u()
            # interleave m0 tail (v'' + transposes) with early m1 units so
            # PE never stalls on the v'' DVE chain before head-pair-0 starts
            fill0 = [m0_tail[0], m1_main[0], m0_tail[1], m1_main[1],
                     m0_tail[2], m1_main[2], m0_tail[3], m1_main[3],
                     m0_tail[4], m0_tail[5]] + m1_main[4:] + m1_tail
            pending = attention(0, fill0)
            for fn in pending:
                fn()
            pending = attention(1, [oc_unit])
            for fn in pending:
                fn()
            for tq in range(12, 16):
                proj_group(tq)

            if dbg:
                for nm, t in [("dq", qT), ("dk", kT), ("dkp", kTp),
                              ("dvp", vpT), ("dy", yT)]:
                    dd = nc.dram_tensor(nm, list(t.shape), BF16,
                                        kind="ExternalOutput")
                    nc.sync.dma_start(dd[:], t[:])
                dvn = nc.dram_tensor("dvn", list(vnat.shape), BF16,
                                     kind="ExternalOutput")
                nc.sync.dma_start(dvn[:], vnat[:])

    nc.finalize()
    return nc


def _host_inputs(x, w_attn, b_attn, w_proj, p_param):
    """Build the 8 per-core input dicts."""
    bf16 = ml_dtypes.bfloat16
    ident = np.eye(P, dtype=np.float32)
    xx = np.arange(P, dtype=np.int64)[:, None]
    yy = np.arange(P, dtype=np.int64)[None, :]
    tri = (yy - xx >= 0).astype(np.float32)
    cst = np.concatenate([ident, tri, tri], axis=1).astype(bf16)
    onesr = np.ones((1, 64), dtype=np.float32)

    def warr(w):  # [C, n] -> [P, KC, n] contiguous
        n = w.shape[1]
        return np.ascontiguousarray(
            w.reshape(KC, P, n).transpose(1, 0, 2)).astype(bf16)

    # x^T quarters: [P, NQ, KC, 512]
    xts = []
    for b in range(B):
        xt = x[b].T.reshape(KC, P, NQ, 512)
        xts.append(np.ascontiguousarray(xt.transpose(1, 2, 0, 3)).astype(bf16))
    in_maps = []
    for core in range(8):
        b, hg = divmod(core, 4)
        cs = slice(hg * CL, (hg + 1) * CL)
        csC = slice(C + hg * CL, C + (hg + 1) * CL)
        cs2C = slice(2 * C + hg * CL, 2 * C + (hg + 1) * CL)
        in_maps.append({
            "xt": xts[b],
            "wq": warr(w_attn[:, cs]),
            "wk": warr(w_attn[:, csC]),
            "wv": warr(w_attn[:, cs2C]),
            "wp": np.ascontiguousarray(
                w_proj[cs, :].reshape(2, P, C).transpose(1, 0, 2)).astype(bf16),
            "bps": np.ascontiguousarray(np.concatenate([
                b_attn[cs].reshape(2, P).T,
                b_attn[csC].reshape(2, P).T,
                (b_attn[cs2C] + SHIFT).reshape(2, P).T,
                p_param[cs].reshape(2, P).T,
            ], axis=1).astype(np.float32)),
            "cst": cst,
            "onesr": onesr,
        })
    return in_maps


def kernel(x, w_attn, b_attn, w_proj, b_proj, p_param, _trace=False):
    x = np.asarray(x, dtype=np.float32)
    w_attn = np.asarray(w_attn, dtype=np.float32)
    b_attn = np.asarray(b_attn, dtype=np.float32)
    w_proj = np.asarray(w_proj, dtype=np.float32)
    b_proj = np.asarray(b_proj, dtype=np.float32)
    p_param = np.asarray(p_param, dtype=np.float32)

    # p == 1 admits a cheaper final transform (no per-tile ln/exp)
    p_eff = np.sign(np.sign(p_param) + 0.5) * np.clip(np.abs(p_param),
                                                      P_MIN, P_MAX)
    fast_p1 = bool(np.all(p_eff == 1.0))

    key = ("nc", fast_p1)
    if key not in _CACHE:
        _CACHE[key] = _build(fast_p1)
    nc = _CACHE[key]

    in_maps = _host_inputs(x, w_attn, b_attn, w_proj, p_param)
    res = run_bass_kernel_spmd(nc, in_maps, core_ids=list(range(8)),
                               trace=_trace)
    _CACHE["last_result"] = res

    out = np.zeros((B, T, C), dtype=np.float32)
    for core in range(8):
        b = core // 4
        out[b] += res.results[core]["out_p"].astype(np.float32)
        out[b] += res.results[core]["oc"].astype(np.float32).sum(0)
    out += b_proj[None, None, :]
    return out


if __name__ == "__main__":
    rng = np.random.default_rng(0)
    ins = {
        "x": rng.standard_normal((B, T, C), dtype=np.float32),
        "w_attn": (rng.standard_normal((C, 3 * C), dtype=np.float32) * 0.02),
        "b_attn": np.zeros(3 * C, np.float32),
        "w_proj": (rng.standard_normal((C, C), dtype=np.float32) * 0.02),
        "b_proj": np.zeros(C, np.float32),
        "p_param": np.ones(C, np.float32),
    }
    out = kernel(**ins)
    print("ran, out shape", out.shape, "finite:", np.isfinite(out).all())


# revision 30
# speedup vs baseline: 1.0650x; 1.0160x over previous
"""Trainium2 Bass kernel for nn_CausalGemAttention.

Reference computation (B=2, T=2048, C=1024, H=16, d=64):
    qkv = x @ w_attn + b_attn ; q,k,v = split(qkv)
    p = sign(sign(p_param)+0.5) * clamp(|p_param|, 1e-4, 1e3)
    vc = clip(|v + 5|, 1e-10); z = p*ln(vc); zmax = max_T(z); v' = exp(z - zmax)
    att = causal_softmax(q k^T / sqrt(d)); mean = att @ v'
    y = exp((zmax + ln(mean)) / p) - 5 ; out = y @ w_proj + b_proj

Sharding: 8 cores = 2 (batch) x 4 (head groups of 4 heads / 256 channels).
Each core computes qkv for its head group (contraction over full C), local
attention, and a partial projection (w_proj rows of its channels); host sums
the 4 partials per batch and adds b_proj.

v2 structure (single pool scope, cross-phase PSUM ring sharing):
  - PSUM map (8 banks exact): tag 'big' 2x[P,2,512]f32 (qkv accum + scores,
    double-buffered so PE never waits on softmax exp), tag 'pv' 1x[65,2,512]
    (paired PV accum), tag 'x' 2x[P,512] (proj/bc/transposes/oc).
  - attention iterates per KEY TILE with both heads of the pair in one
    scores tile; softmax exp is one strided [P,2,512-off] ACT op; only the
    128-wide diagonal block needs a triangular mask ([P,2,128] DVE mult).
  - v'' is pre-scaled by exp(zmax) per channel, and the constant term
    (exp(zmax)*cmid - 5) is folded into a per-core [2,C] vector
    oc = const^T @ w_proj (hi/lo bf16 split) added on host. The pair
    post-chain is: den copy (partition 64->0), fast reciprocal, F32R
    rounding copy, me evacuation, two K=1 broadcast matmuls + multiplies.
  - scores moving operand is raw qT (both heads packed); only the stationary
    k is zero-padded to K=128 (kTp), so no padded q buffer is needed.
  - m=1 qkv/transform units are emitted interleaved into head-pair-0
    attention (between exp and PV) to fill PE gaps; projection and
    post-chains trail one q-block behind.
  - output partials are bf16 (halves the DMA-out + PSUM evacuation cost).
"""

import sys
sys.path.insert(0, "/opt/trn_rl_repo")

import numpy as np
import ml_dtypes

import concourse.bacc as bacc
import concourse.tile as tile
from concourse import mybir
from concourse.bass_utils import run_bass_kernel_spmd

F32 = mybir.dt.float32
F32R = mybir.dt.float32r
BF16 = mybir.dt.bfloat16
AF = mybir.ActivationFunctionType
ALU = mybir.AluOpType
AX = mybir.AxisListType

B, T, C, H, D = 2, 2048, 1024, 16, 64
P = 128
CL = 256            # channels per core (4 heads x 64)
KC = C // P         # 8 contraction chunks for qkv
NQ = T // 512       # 4 query blocks of 512
NK = T // P         # 16 key tiles of 128
SHIFT = 5.0
P_MIN, P_MAX, V_MIN = 1e-4, 1e3, 1e-10
SM_SCALE = 1.0 / 8.0  # 1/sqrt(64)

# cst layout (bf16): [ident128 | tri x2]
CST_W = 128 + 256

_CACHE = {}


def _build(fast_p1, dbg=False):
    nc = bacc.Bacc("TRN2", target_bir_lowering=False, debug=False)

    xt_d = nc.dram_tensor("xt", [P, NQ, KC, 512], BF16,
                          kind="ExternalInput")
    wq_d = nc.dram_tensor("wq", [P, KC, CL], BF16, kind="ExternalInput")
    wk_d = nc.dram_tensor("wk", [P, KC, CL], BF16, kind="ExternalInput")
    wv_d = nc.dram_tensor("wv", [P, KC, CL], BF16, kind="ExternalInput")
    wp_d = nc.dram_tensor("wp", [P, 2, C], BF16, kind="ExternalInput")
    # bps = [bq(2) | bk(2) | bv5(2) | pp(2)] per chunk, fp32
    bps_d = nc.dram_tensor("bps", [P, 8], F32, kind="ExternalInput")
    cst_d = nc.dram_tensor("cst", [P, CST_W], BF16, kind="ExternalInput")
    or_d = nc.dram_tensor("onesr", [1, 64], F32R, kind="ExternalInput")
    out_d = nc.dram_tensor("out_p", [T, C], BF16, kind="ExternalOutput")
    oc_d = nc.dram_tensor("oc", [2, C], F32, kind="ExternalOutput")

    with tile.TileContext(nc) as tc:
        with (
            tc.tile_pool(name="consts", bufs=1) as cp,
            tc.tile_pool(name="main", bufs=1) as mp,
            tc.tile_pool(name="ptp", bufs=6) as ptp,
            tc.tile_pool(name="sm", bufs=2) as smp,
            tc.tile_pool(name="outp", bufs=3) as opp,
            tc.tile_pool(name="psBig", bufs=2, space="PSUM") as psBig,
            tc.tile_pool(name="psPv", bufs=1, space="PSUM") as psPv,
            tc.tile_pool(name="psX", bufs=2, space="PSUM") as psX,
        ):
            # ---------------- constants + inputs ------------------------------
            cst = cp.tile([P, CST_W], BF16)
            onesr = cp.tile([1, 64], F32R)
            bps = cp.tile([P, 8], F32)
            nc.scalar.dma_start(bps[:], bps_d[:])
            nc.scalar.dma_start(cst[:], cst_d[:])
            nc.scalar.dma_start(onesr[:], or_d[:])
            ident = cst[:, 0:128]
            tri2 = cst[:, 128:384].rearrange("p (b c) -> p b c", b=2)
            bq_sb = bps[:, 0:2]
            bk_sb = bps[:, 2:4]
            bv5_sb = bps[:, 4:6]
            pp_sb = bps[:, 6:8]

            wq_sb = mp.tile([P, KC, CL], BF16)
            wk_sb = mp.tile([P, KC, CL], BF16)
            wv_sb = mp.tile([P, KC, CL], BF16)
            wp_sb = mp.tile([P, 2, C], BF16)
            xt_sb = mp.tile([P, NQ, KC, 512], BF16)
            # all inputs are host-prearranged contiguous; x in query-block
            # quarters so the first v matmul group starts ~7us in
            # wave 1: only what the first query quarter needs, one piece
            # per trigger queue; later quarters are triggered from the
            # SCALAR queue between the per-quarter |v| ops, so they enter
            # the (round-robin) DMA queue only as the previous quarter is
            # being consumed and don't dilute wave-1 bandwidth.
            nc.scalar.dma_start(wv_sb[:], wv_d[:])
            nc.scalar.dma_start(xt_sb[:, 0, 0:4], xt_d[:, 0, 0:4])
            nc.scalar.dma_start(xt_sb[:, 0, 4:8], xt_d[:, 0, 4:8])
            nc.gpsimd.dma_start(wq_sb[:], wq_d[:])
            nc.gpsimd.dma_start(wk_sb[:], wk_d[:])

            qT = mp.tile([P, 2, T], BF16)    # q^T: [c%128, c//128, t]
            kT = mp.tile([P, 2, T], BF16)
            # padded per-head k (stationary side only): head h occupies
            # partitions 64*(h%2):64*(h%2)+64, rest zero
            kTp = mp.tile([P, 4, T], BF16)
            vT = mp.tile([P, 2, T], F32)
            vpT = mp.tile([P, 2, T], BF16)
            vnat = mp.tile([P, 4, NK, 65], BF16)  # [tk%128, head, tk//128, d|1]
            yT = mp.tile([P, 2, T], BF16)
            nc.gpsimd.memset(kTp[:], 0.0)
            for h in range(4):
                nc.vector.memset(vnat[:, h, :, 64], 1.0)

            # p = sign(sign(pp)+0.5) * clamp(|pp|, P_MIN, P_MAX)
            sgn = cp.tile([P, 2], F32)
            ab = cp.tile([P, 2], F32)
            p_sb = cp.tile([P, 2], F32)
            # allcp packs [ip | zmaxp | cmid | ezp | ecp5 | eip] x 2 chunks
            allcp = cp.tile([P, 6, 2], F32)
            ip_sb = allcp[:, 0, :]
            zmaxp = allcp[:, 1, :]
            cmid = allcp[:, 2, :]
            ezp = allcp[:, 3, :]
            ecp5 = allcp[:, 4, :]
            eip = allcp[:, 5, :]
            nc.scalar.activation(sgn[:], pp_sb[:], AF.Sign)
            nc.vector.tensor_scalar_add(sgn[:], sgn[:], 0.5)
            nc.scalar.activation(sgn[:], sgn[:], AF.Sign)
            nc.scalar.activation(ab[:], pp_sb[:], AF.Abs)
            nc.vector.tensor_scalar(ab[:], ab[:], float(P_MIN), float(P_MAX),
                                    ALU.max, ALU.min)
            nc.vector.tensor_tensor(p_sb[:], sgn[:], ab[:], ALU.mult)
            nc.vector.reciprocal(ip_sb[:], p_sb[:])

            negzmax = cp.tile([P, 2], F32)
            zmin_sb = cp.tile([P, 2], F32)
            pmax = cp.tile([P, 2, NQ], F32)   # per-quarter -max partials
            pmin = cp.tile([P, 2, NQ], F32)
            # per-head [64,1] base-0 views (general-p path)
            allh = cp.tile([64, 6, 4], F32)
            iph = allh[:, 0, :]
            zmh = allh[:, 1, :]
            cmh = allh[:, 2, :]
            eih = allh[:, 5, :]
            ecb = cp.tile([P, 2, 2], BF16)  # ecp5 (or -SHIFT) hi/lo for oc
            ecl = cp.tile([P, 2], F32)

            # ---------------- emission units ----------------------------------
            def qkv_group(wsb, kind, m, nt):
                ps = psBig.tile([P, 512], F32, tag="big", name="ev")
                for kc in range(KC):
                    nc.tensor.matmul(
                        ps[:],
                        wsb[:, kc, m * P:(m + 1) * P],
                        xt_sb[:, nt, kc, :],
                        start=(kc == 0), stop=(kc == KC - 1),
                    )
                tsl = slice(nt * 512, (nt + 1) * 512)
                if kind == "q":
                    nc.vector.tensor_scalar_add(
                        qT[:, m, tsl], ps[:], bq_sb[:, m:m + 1])
                elif kind == "k":
                    nc.vector.tensor_scalar_add(
                        kT[:, m, tsl], ps[:], bk_sb[:, m:m + 1])
                    if m == 0:
                        for h in (0, 1):
                            base = 64 * (h % 2)
                            nc.scalar.dma_start(kTp[base:base + 64, h, tsl],
                                                kT[base:base + 64, m, tsl])
                else:
                    # fast (p=1): vT = |v + b + SHIFT| directly; the power-mean
                    # collapses to a plain max-normalized mean so no ln/exp is
                    # needed (keeps ScalarE on one activation table set).
                    # general: z = p * ln(|v + b + SHIFT|)
                    nc.scalar.activation(
                        vT[:, m, tsl], ps[:], AF.Abs,
                        bias=bv5_sb[:, m:m + 1])
                    if m == 0 and nt < NQ - 1:
                        nc.scalar.dma_start(xt_sb[:, nt + 1, 0:4],
                                            xt_d[:, nt + 1, 0:4])
                        nc.scalar.dma_start(xt_sb[:, nt + 1, 4:8],
                                            xt_d[:, nt + 1, 4:8])
                    elif m == 0:
                        nc.scalar.dma_start(wp_sb[:], wp_d[:])
                    if not fast_p1:
                        nc.scalar.activation(vT[:, m, tsl], vT[:, m, tsl],
                                             AF.Ln)
                        nc.vector.tensor_scalar_mul(
                            vT[:, m, tsl], vT[:, m, tsl], p_sb[:, m:m + 1])
                    nc.vector.tensor_reduce(pmax[:, m, nt:nt + 1],
                                            vT[:, m, tsl], AX.X,
                                            op=ALU.max, negate=True)
                    nc.vector.tensor_reduce(pmin[:, m, nt:nt + 1],
                                            vT[:, m, tsl], AX.X, op=ALU.min)

            def v_tail(m):
                nc.vector.tensor_reduce(negzmax[:, m:m + 1], pmax[:, m, :],
                                        AX.X, op=ALU.min)
                nc.vector.tensor_reduce(zmin_sb[:, m:m + 1], pmin[:, m, :],
                                        AX.X, op=ALU.min)
                if fast_p1:
                    # v''*ezp = |v+5| - 0.5*(max+min); ecp5 = 0.5*(max+min)-5
                    # co = 0.5*(max+min) stored in ecp5 slot (pre -SHIFT)
                    nc.vector.scalar_tensor_tensor(
                        ecp5[:, m:m + 1], negzmax[:, m:m + 1], -1.0,
                        zmin_sb[:, m:m + 1], ALU.mult, ALU.add)
                    nc.vector.tensor_scalar_mul(ecp5[:, m:m + 1],
                                                ecp5[:, m:m + 1], 0.5)
                    nc.vector.tensor_scalar_sub(vpT[:, m, :], vT[:, m, :],
                                                ecp5[:, m:m + 1])
                    nc.vector.tensor_scalar_add(ecp5[:, m:m + 1],
                                                ecp5[:, m:m + 1], -SHIFT)
                    return
                # general p: zmax/zmin; cmid = 0.5*(1+exp(zmin-zmax));
                # ezp = exp(zmax); ecp5 = ezp*cmid - 5; v'' = (v'-cmid)*ezp
                nc.vector.scalar_tensor_tensor(
                    zmaxp[:, m:m + 1], negzmax[:, m:m + 1], -1.0,
                    ip_sb[:, m:m + 1], ALU.mult, ALU.mult)
                nc.scalar.activation(cmid[:, m:m + 1], zmin_sb[:, m:m + 1],
                                     AF.Exp, bias=negzmax[:, m:m + 1])
                nc.vector.tensor_scalar(cmid[:, m:m + 1], cmid[:, m:m + 1],
                                        1.0, 0.5, ALU.add, ALU.mult)
                nc.scalar.activation(ezp[:, m:m + 1], negzmax[:, m:m + 1],
                                     AF.Exp, scale=-1.0)
                nc.scalar.activation(eip[:, m:m + 1], negzmax[:, m:m + 1],
                                     AF.Exp)
                nc.vector.scalar_tensor_tensor(
                    ecp5[:, m:m + 1], ezp[:, m:m + 1], 0.0,
                    cmid[:, m:m + 1], ALU.bypass, ALU.mult)
                nc.vector.tensor_scalar_add(ecp5[:, m:m + 1], ecp5[:, m:m + 1],
                                            -SHIFT)
                nc.scalar.activation(vT[:, m, :], vT[:, m, :], AF.Exp,
                                     bias=negzmax[:, m:m + 1])
                nc.vector.tensor_scalar(vpT[:, m, :], vT[:, m, :],
                                        cmid[:, m:m + 1], ezp[:, m:m + 1],
                                        ALU.subtract, ALU.mult)

            def kTp_fill(m):
                if m == 0:
                    return   # filled per-quarter from the k bias-adds
                for h in (2 * m, 2 * m + 1):
                    base = 64 * (h % 2)
                    nc.gpsimd.dma_start(kTp[base:base + 64, h, :],
                                        kT[base:base + 64, m, :])

            def transpose_unit(m, tp):
                # 4 key tiles -> vnat via one [128,128] PE transpose each
                trp = psX.tile([P, 512], BF16, tag="x", name="trp")
                for j in range(4):
                    kt = 4 * tp + j
                    nc.tensor.transpose(
                        trp[:, j * 128:(j + 1) * 128],
                        vpT[:, m, kt * P:(kt + 1) * P],
                        ident,
                    )
                t3 = trp[:].rearrange("p (a b) -> p a b", a=4)
                nc.vector.tensor_copy(
                    vnat[:, 2 * m, 4 * tp:4 * tp + 4, 0:64], t3[:, :, 0:64])
                nc.vector.tensor_copy(
                    vnat[:, 2 * m + 1, 4 * tp:4 * tp + 4, 0:64],
                    t3[:, :, 64:128])

            def head_consts(m):
                nc.sync.dma_start(allh[:, :, 2 * m], allcp[0:64, :, m])
                nc.sync.dma_start(allh[:, :, 2 * m + 1], allcp[64:128, :, m])

            def m_units(m):
                # main: feed-rate-matched (v,q,k per query quarter) + kTp;
                # tail: v-transform + transposes (deferred into attention
                # fillers so scores can start as soon as kTp is ready)
                units = []
                for nt in range(NQ):
                    units.append(lambda m=m, nt=nt: qkv_group(wv_sb, "v", m, nt))
                    units.append(lambda m=m, nt=nt: qkv_group(wq_sb, "q", m, nt))
                    units.append(lambda m=m, nt=nt: qkv_group(wk_sb, "k", m, nt))
                units.append(lambda m=m: kTp_fill(m))
                tail = [lambda m=m: v_tail(m)]
                for tp in range(4):
                    tail.append(lambda m=m, tp=tp: transpose_unit(m, tp))
                tail.append(lambda m=m: head_consts(m))
                return units, tail

            # ---------------- attention ---------------------------------------
            def post_pair(pv_t, hp, qi):
                # yh = num'' / den per head; const term folded into oc
                dcp = smp.tile([1, 1024], F32, tag="dcp", name="dcp")
                nc.vector.tensor_copy(dcp[:], pv_t[64:65, :, :])
                rdf = smp.tile([1, 1024], F32, tag="rdf", name="rdf")
                nc.vector.reciprocal_approx_fast(rdf[:], dcp[:])
                rd = smp.tile([1, 1024], F32R, tag="rd", name="rd")
                nc.vector.tensor_copy(rd[:], rdf[:])
                me_s = smp.tile([64, 2, 512], F32, tag="me_s",
                                name="me_s")
                nc.vector.tensor_copy(me_s[:], pv_t[0:64, :, :])
                yh = smp.tile([64, 2, 512], BF16, tag="yh", name="yh")
                for i in range(2):
                    h = 2 * hp + i
                    bc = psX.tile([64, 512], F32, tag="x", name="bc",
                                  padded_shape=[P, 512])
                    nc.tensor.matmul(bc[:], onesr[:],
                                     rd[:, i * 512:(i + 1) * 512],
                                     start=True, stop=True)
                    if fast_p1:
                        nc.vector.tensor_tensor(yh[:, i, :], me_s[:, i, :],
                                                bc[:], ALU.mult)
                    else:
                        me = smp.tile([64, 512], F32, tag="me", name="me")
                        nc.vector.tensor_tensor(me[:], me_s[:, i, :], bc[:],
                                                ALU.mult)
                        nc.vector.tensor_scalar(me[:], me[:],
                                                eih[:, h:h + 1],
                                                cmh[:, h:h + 1],
                                                ALU.mult, ALU.add)
                        nc.scalar.activation(me[:], me[:], AF.Ln)
                        nc.scalar.activation(yh[:, i, :], me[:], AF.Exp,
                                             scale=iph[:, h:h + 1],
                                             bias=zmh[:, h:h + 1])
                if dbg and hp == 0 and qi == 0:
                    dpv = cp.tile([65, 1024], F32, name="dpv")
                    nc.vector.tensor_copy(
                        dpv[:], pv_t[:].rearrange("p a b -> p (a b)"))
                    dpv_d = nc.dram_tensor("dpv", [65, 1024], F32,
                                           kind="ExternalOutput")
                    nc.sync.dma_start(dpv_d[:], dpv[:])
                    dyh_d = nc.dram_tensor("dyh", [64, 1024], BF16,
                                           kind="ExternalOutput")
                    nc.sync.dma_start(dyh_d[:],
                                      yh[:].rearrange("p a b -> p (a b)"))
                    drd_d = nc.dram_tensor("drd", [2, 1024], F32,
                                           kind="ExternalOutput")
                    nc.sync.dma_start(drd_d[0:1, :], rdf[:])
                    nc.sync.dma_start(drd_d[1:2, :], dcp[:])
                qi_ = post_qi[0]
                qsl = slice(qi_ * 512, (qi_ + 1) * 512)
                nc.gpsimd.dma_start(yT[0:64, hp, qsl], yh[:, 0, :])
                nc.gpsimd.dma_start(yT[64:128, hp, qsl], yh[:, 1, :])

            post_qi = [0]

            def proj_group(tq):
                po = opp.tile([P, C], BF16, tag="po", name="po")
                for nh in range(2):
                    pj = psX.tile([P, 512], F32, tag="x", name="pj")
                    for c in range(2):
                        nc.tensor.matmul(
                            pj[:],
                            yT[:, c, tq * P:(tq + 1) * P],
                            wp_sb[:, c, nh * 512:(nh + 1) * 512],
                            start=(c == 0), stop=(c == 1),
                        )
                    nc.vector.tensor_copy(po[:, nh * 512:(nh + 1) * 512],
                                          pj[:])
                    eng = nc.gpsimd if nh == 0 else nc.sync
                    eng.dma_start(
                        out_d[tq * P:(tq + 1) * P, nh * 512:(nh + 1) * 512],
                        po[:, nh * 512:(nh + 1) * 512])

            def oc_unit():
                # oc = const^T @ w_proj, const = ecp5 (fast) or -SHIFT
                # (general), hi/lo split across two bf16 stationary columns
                if fast_p1:
                    nc.vector.tensor_copy(ecb[:, :, 0], ecp5[:])
                    nc.vector.tensor_tensor(ecl[:], ecp5[:], ecb[:, :, 0],
                                            ALU.subtract)
                    nc.vector.tensor_copy(ecb[:, :, 1], ecl[:])
                else:
                    nc.vector.memset(ecb[:], 0.0)
                    nc.vector.memset(ecb[:, :, 0], -SHIFT)
                occ = cp.tile([2, C], F32)
                for nh in range(2):
                    ocp = psX.tile([2, 512], F32, tag="x", name="ocp",
                                   padded_shape=[P, 512])
                    for c in range(2):
                        nc.tensor.matmul(
                            ocp[:], ecb[:, c, :],
                            wp_sb[:, c, nh * 512:(nh + 1) * 512],
                            start=(c == 0), stop=(c == 1),
                        )
                    nc.vector.tensor_copy(occ[:, nh * 512:(nh + 1) * 512],
                                          ocp[:])
                nc.sync.dma_start(oc_d[:], occ[:])

            def attention(hp, fillers):
                pending = []
                fill_i = [0]
                n_iters = sum(4 * qi + 4 for qi in range(NQ))
                it = [0]

                def pace():
                    want = min(len(fillers),
                               3 + it[0] * max(len(fillers) - 3, 0) // n_iters)
                    while fill_i[0] < want:
                        fillers[fill_i[0]]()
                        fill_i[0] += 1

                for qi in range(NQ):
                    nkt = 4 * qi + 4
                    pv = psPv.tile([65, 2, 512], F32, tag="pv", name="pv")
                    prev = None
                    for kt in range(nkt):
                        it[0] += 1
                        off = P * max(kt - 4 * qi, 0)
                        s = psBig.tile([P, 2, 512], F32, tag="big", name="s")
                        for i in range(2):
                            h = 2 * hp + i
                            nc.tensor.matmul(
                                s[:, i, off:512],
                                kTp[:, h, kt * P:(kt + 1) * P],
                                qT[:, hp, qi * 512 + off:(qi + 1) * 512],
                                start=True, stop=True,
                                skip_group_check=True,
                            )
                        pt = ptp.tile([P, 2, 512], BF16, tag="pt", name="pt")
                        nc.scalar.activation(pt[:, :, off:512],
                                             s[:, :, off:512],
                                             AF.Exp, scale=SM_SCALE)
                        if kt >= 4 * qi:   # diagonal: mask the 128-block
                            nc.vector.tensor_mul(pt[:, :, off:off + P],
                                                 pt[:, :, off:off + P],
                                                 tri2[:])
                        if dbg and hp == 0 and qi == 0 and kt == 0:
                            dpt_d = nc.dram_tensor("dpt", [P, 1024], BF16,
                                                   kind="ExternalOutput")
                            nc.sync.dma_start(
                                dpt_d[:], pt[:].rearrange("p a b -> p (a b)"))
                            ds_t = cp.tile([P, 1024], F32, name="ds_t")
                            nc.vector.tensor_copy(
                                ds_t[:], s[:].rearrange("p a b -> p (a b)"))
                            ds_d = nc.dram_tensor("ds", [P, 1024], F32,
                                                  kind="ExternalOutput")
                            nc.sync.dma_start(ds_d[:], ds_t[:])
                        pace()
                        if kt == 1:
                            for fn in pending:
                                fn()
                            pending.clear()
                            if hp == 1 and qi > 0:
                                for tq in range(4 * (qi - 1), 4 * qi):
                                    proj_group(tq)
                        if prev is not None:
                            pkt, ppt = prev
                            o_ = P * max(pkt - 4 * qi, 0)
                            for i in range(2):
                                h = 2 * hp + i
                                nc.tensor.matmul(
                                    pv[:, i, o_:512],
                                    vnat[:, h, pkt, :],
                                    ppt[:, i, o_:512],
                                    start=(pkt == 0), stop=(pkt == nkt - 1),
                                    skip_group_check=True,
                                )
                        prev = (kt, pt)
                    pkt, ppt = prev
                    o_ = P * max(pkt - 4 * qi, 0)
                    for i in range(2):
                        h = 2 * hp + i
                        nc.tensor.matmul(
                            pv[:, i, o_:512],
                            vnat[:, h, pkt, :],
                            ppt[:, i, o_:512],
                            start=(pkt == 0), stop=(pkt == nkt - 1),
                            skip_group_check=True,
                        )

                    def mk(pv_t, hp_, qi_):
                        def fn():
                            post_qi[0] = qi_
                            post_pair(pv_t, hp_, qi_)
                        return fn
                    pending.append(mk(pv, hp, qi))
                while fill_i[0] < len(fillers):
                    fillers[fill_i[0]]()
                    fill_i[0] += 1
                return pending

            # ---------------- schedule ----------------------------------------
            m0_main, m0_tail = m_units(0)
            m1_main, m1_tail = m_units(1)
            for u in m0_main:
                u()
            # interleave m0 tail (v'' + transposes) with early m1 units so
            # PE never stalls on the v'' DVE chain before head-pair-0 starts
            fill0 = [m0_tail[0], m1_main[0], m0_tail[1], m1_main[1],
                     m0_tail[2], m1_main[2], m0_tail[3], m1_main[3],
                     m0_tail[4], m0_tail[5]] + m1_main[4:] + m1_tail
            pending = attention(0, fill0)
            for fn in pending:
                fn()
            pending = attention(1, [oc_unit])
            for fn in pending:
                fn()
            for tq in range(12, 16):
                proj_group(tq)

            if dbg:
                for nm, t in [("dq", qT), ("dk", kT), ("dkp", kTp),
                              ("dvp", vpT), ("dy", yT)]:
                    dd = nc.dram_tensor(nm, list(t.shape), BF16,
                                        kind="ExternalOutput")
                    nc.sync.dma_start(dd[:], t[:])
                dvn = nc.dram_tensor("dvn", list(vnat.shape), BF16,
                                     kind="ExternalOutput")
                nc.sync.dma_start(dvn[:], vnat[:])

    nc.finalize()
    return nc


def _host_inputs(x, w_attn, b_attn, w_proj, p_param):
    """Build the 8 per-core input dicts."""
    bf16 = ml_dtypes.bfloat16
    ident = np.eye(P, dtype=np.float32)
    xx = np.arange(P, dtype=np.int64)[:, None]
    yy = np.arange(P, dtype=np.int64)[None, :]
    tri = (yy - xx >= 0).astype(np.float32)
    cst = np.concatenate([ident, tri, tri], axis=1).astype(bf16)
    onesr = np.ones((1, 64), dtype=np.float32)

    def warr(w):  # [C, n] -> [P, KC, n] contiguous
        n = w.shape[1]
        return np.ascontiguousarray(
            w.reshape(KC, P, n).transpose(1, 0, 2)).astype(bf16)

    # x^T quarters: [P, NQ, KC, 512]
    xts = []
    for b in range(B):
        xt = x[b].T.reshape(KC, P, NQ, 512)
        xts.append(np.ascontiguousarray(xt.transpose(1, 2, 0, 3)).astype(bf16))
    in_maps = []
    for core in range(8):
        b, hg = divmod(core, 4)
        cs = slice(hg * CL, (hg + 1) * CL)
        csC = slice(C + hg * CL, C + (hg + 1) * CL)
        cs2C = slice(2 * C + hg * CL, 2 * C + (hg + 1) * CL)
        in_maps.append({
            "xt": xts[b],
            "wq": warr(w_attn[:, cs]),
            "wk": warr(w_attn[:, csC]),
            "wv": warr(w_attn[:, cs2C]),
            "wp": np.ascontiguousarray(
                w_proj[cs, :].reshape(2, P, C).transpose(1, 0, 2)).astype(bf16),
            "bps": np.ascontiguousarray(np.concatenate([
                b_attn[cs].reshape(2, P).T,
                b_attn[csC].reshape(2, P).T,
                (b_attn[cs2C] + SHIFT).reshape(2, P).T,
                p_param[cs].reshape(2, P).T,
            ], axis=1).astype(np.float32)),
            "cst": cst,
            "onesr": onesr,
        })
    return in_maps


def kernel(x, w_attn, b_attn, w_proj, b_proj, p_param, _trace=False):
    x = np.asarray(x, dtype=np.float32)
    w_attn = np.asarray(w_attn, dtype=np.float32)
    b_attn = np.asarray(b_attn, dtype=np.float32)
    w_proj = np.asarray(w_proj, dtype=np.float32)
    b_proj = np.asarray(b_proj, dtype=np.float32)
    p_param = np.asarray(p_param, dtype=np.float32)

    # p == 1 admits a cheaper final transform (no per-tile ln/exp)
    p_eff = np.sign(np.sign(p_param) + 0.5) * np.clip(np.abs(p_param),
                                                      P_MIN, P_MAX)
    fast_p1 = bool(np.all(p_eff == 1.0))

    key = ("nc", fast_p1)
    if key not in _CACHE:
        _CACHE[key] = _build(fast_p1)
    nc = _CACHE[key]

    in_maps = _host_inputs(x, w_attn, b_attn, w_proj, p_param)
    res = run_bass_kernel_spmd(nc, in_maps, core_ids=list(range(8)),
                               trace=_trace)
    _CACHE["last_result"] = res

    out = np.zeros((B, T, C), dtype=np.float32)
    for core in range(8):
        b = core // 4
        out[b] += res.results[core]["out_p"].astype(np.float32)
        out[b] += res.results[core]["oc"].astype(np.float32).sum(0)
    out += b_proj[None, None, :]
    return out


if __name__ == "__main__":
    rng = np.random.default_rng(0)
    ins = {
        "x": rng.standard_normal((B, T, C), dtype=np.float32),
        "w_attn": (rng.standard_normal((C, 3 * C), dtype=np.float32) * 0.02),
        "b_attn": np.zeros(3 * C, np.float32),
        "w_proj": (rng.standard_normal((C, C), dtype=np.float32) * 0.02),
        "b_proj": np.zeros(C, np.float32),
        "p_param": np.ones(C, np.float32),
    }
    out = kernel(**ins)
    print("ran, out shape", out.shape, "finite:", np.isfinite(out).all())


# revision 31
# speedup vs baseline: 1.0804x; 1.0145x over previous
"""Trainium2 Bass kernel for nn_CausalGemAttention.

Reference computation (B=2, T=2048, C=1024, H=16, d=64):
    qkv = x @ w_attn + b_attn ; q,k,v = split(qkv)
    p = sign(sign(p_param)+0.5) * clamp(|p_param|, 1e-4, 1e3)
    vc = clip(|v + 5|, 1e-10); z = p*ln(vc); zmax = max_T(z); v' = exp(z - zmax)
    att = causal_softmax(q k^T / sqrt(d)); mean = att @ v'
    y = exp((zmax + ln(mean)) / p) - 5 ; out = y @ w_proj + b_proj

Sharding: 8 cores = 2 (batch) x 4 (head groups of 4 heads / 256 channels).
Each core computes qkv for its head group (contraction over full C), local
attention, and a partial projection (w_proj rows of its channels); host sums
the 4 partials per batch and adds b_proj.

v2 structure (single pool scope, cross-phase PSUM ring sharing):
  - PSUM map (8 banks exact): tag 'big' 2x[P,2,512]f32 (qkv accum + scores,
    double-buffered so PE never waits on softmax exp), tag 'pv' 1x[65,2,512]
    (paired PV accum), tag 'x' 2x[P,512] (proj/bc/transposes/oc).
  - attention iterates per KEY TILE with both heads of the pair in one
    scores tile; softmax exp is one strided [P,2,512-off] ACT op; only the
    128-wide diagonal block needs a triangular mask ([P,2,128] DVE mult).
  - v'' is pre-scaled by exp(zmax) per channel, and the constant term
    (exp(zmax)*cmid - 5) is folded into a per-core [2,C] vector
    oc = const^T @ w_proj (hi/lo bf16 split) added on host. The pair
    post-chain is: den copy (partition 64->0), fast reciprocal, F32R
    rounding copy, me evacuation, two K=1 broadcast matmuls + multiplies.
  - scores moving operand is raw qT (both heads packed); only the stationary
    k is zero-padded to K=128 (kTp), so no padded q buffer is needed.
  - m=1 qkv/transform units are emitted interleaved into head-pair-0
    attention (between exp and PV) to fill PE gaps; projection and
    post-chains trail one q-block behind.
  - output partials are bf16 (halves the DMA-out + PSUM evacuation cost).
"""

import sys
sys.path.insert(0, "/opt/trn_rl_repo")

import numpy as np
import ml_dtypes

import concourse.bacc as bacc
import concourse.tile as tile
from concourse import mybir
from concourse.bass_utils import run_bass_kernel_spmd

F32 = mybir.dt.float32
F32R = mybir.dt.float32r
BF16 = mybir.dt.bfloat16
AF = mybir.ActivationFunctionType
ALU = mybir.AluOpType
AX = mybir.AxisListType

B, T, C, H, D = 2, 2048, 1024, 16, 64
P = 128
CL = 256            # channels per core (4 heads x 64)
KC = C // P         # 8 contraction chunks for qkv
NQ = T // 512       # 4 query blocks of 512
NK = T // P         # 16 key tiles of 128
SHIFT = 5.0
P_MIN, P_MAX, V_MIN = 1e-4, 1e3, 1e-10
SM_SCALE = 1.0 / 8.0  # 1/sqrt(64)

# cst layout (bf16): [ident128 | tri x2]
CST_W = 128 + 256

_CACHE = {}


def _build(fast_p1, dbg=False):
    nc = bacc.Bacc("TRN2", target_bir_lowering=False, debug=False)

    xt_d = nc.dram_tensor("xt", [P, NQ, KC, 512], BF16,
                          kind="ExternalInput")
    wq_d = nc.dram_tensor("wq", [P, KC, CL], BF16, kind="ExternalInput")
    wk_d = nc.dram_tensor("wk", [P, KC, CL], BF16, kind="ExternalInput")
    wv_d = nc.dram_tensor("wv", [P, KC, CL], BF16, kind="ExternalInput")
    wp_d = nc.dram_tensor("wp", [P, 2, C], BF16, kind="ExternalInput")
    # bps = [bq(2) | bk(2) | bv5(2) | pp(2)] per chunk, fp32
    bps_d = nc.dram_tensor("bps", [P, 8], F32, kind="ExternalInput")
    cst_d = nc.dram_tensor("cst", [P, CST_W], BF16, kind="ExternalInput")
    or_d = nc.dram_tensor("onesr", [1, 64], F32R, kind="ExternalInput")
    out_d = nc.dram_tensor("out_p", [T, C], BF16, kind="ExternalOutput")
    oc_d = nc.dram_tensor("oc", [2, C], F32, kind="ExternalOutput")

    with tile.TileContext(nc) as tc:
        with (
            tc.tile_pool(name="consts", bufs=1) as cp,
            tc.tile_pool(name="main", bufs=1) as mp,
            tc.tile_pool(name="ptp", bufs=6) as ptp,
            tc.tile_pool(name="sm", bufs=2) as smp,
            tc.tile_pool(name="outp", bufs=3) as opp,
            tc.tile_pool(name="psBig", bufs=2, space="PSUM") as psBig,
            tc.tile_pool(name="psPv", bufs=1, space="PSUM") as psPv,
            tc.tile_pool(name="psX", bufs=2, space="PSUM") as psX,
        ):
            # ---------------- constants + inputs ------------------------------
            cst = cp.tile([P, CST_W], BF16)
            onesr = cp.tile([1, 64], F32R)
            bps = cp.tile([P, 8], F32)
            nc.scalar.dma_start(bps[:], bps_d[:])
            nc.scalar.dma_start(cst[:], cst_d[:])
            nc.scalar.dma_start(onesr[:], or_d[:])
            ident = cst[:, 0:128]
            tri2 = cst[:, 128:384].rearrange("p (b c) -> p b c", b=2)
            bq_sb = bps[:, 0:2]
            bk_sb = bps[:, 2:4]
            bv5_sb = bps[:, 4:6]
            pp_sb = bps[:, 6:8]

            wq_sb = mp.tile([P, KC, CL], BF16)
            wk_sb = mp.tile([P, KC, CL], BF16)
            wv_sb = mp.tile([P, KC, CL], BF16)
            wp_sb = mp.tile([P, 2, C], BF16)
            xt_sb = mp.tile([P, NQ, KC, 512], BF16)
            # all inputs are host-prearranged contiguous; x in query-block
            # quarters so the first v matmul group starts ~7us in
            # wave 1: only what the first query quarter needs, one piece
            # per trigger queue; later quarters are triggered from the
            # SCALAR queue between the per-quarter |v| ops, so they enter
            # the (round-robin) DMA queue only as the previous quarter is
            # being consumed and don't dilute wave-1 bandwidth.
            nc.scalar.dma_start(wv_sb[:], wv_d[:])
            nc.scalar.dma_start(xt_sb[:, 0, 0:4], xt_d[:, 0, 0:4])
            nc.scalar.dma_start(xt_sb[:, 0, 4:8], xt_d[:, 0, 4:8])
            nc.gpsimd.dma_start(wq_sb[:], wq_d[:])
            nc.gpsimd.dma_start(wk_sb[:], wk_d[:])

            qT = mp.tile([P, 2, T], BF16)    # q^T: [c%128, c//128, t]
            kT = mp.tile([P, 2, T], BF16)
            # padded per-head k (stationary side only): head h occupies
            # partitions 64*(h%2):64*(h%2)+64, rest zero
            kTp = mp.tile([P, 4, T], BF16)
            vT = mp.tile([P, 2, T], F32)
            vpT = mp.tile([P, 2, T], BF16)
            vnat = mp.tile([P, 4, NK, 65], BF16)  # [tk%128, head, tk//128, d|1]
            yT = mp.tile([P, 2, T], BF16)
            nc.gpsimd.memset(kTp[:], 0.0)
            for h in range(4):
                nc.vector.memset(vnat[:, h, :, 64], 1.0)

            # p = sign(sign(pp)+0.5) * clamp(|pp|, P_MIN, P_MAX)
            sgn = cp.tile([P, 2], F32)
            ab = cp.tile([P, 2], F32)
            p_sb = cp.tile([P, 2], F32)
            # allcp packs [ip | zmaxp | cmid | ezp | ecp5 | eip] x 2 chunks
            allcp = cp.tile([P, 6, 2], F32)
            ip_sb = allcp[:, 0, :]
            zmaxp = allcp[:, 1, :]
            cmid = allcp[:, 2, :]
            ezp = allcp[:, 3, :]
            ecp5 = allcp[:, 4, :]
            eip = allcp[:, 5, :]
            nc.scalar.activation(sgn[:], pp_sb[:], AF.Sign)
            nc.vector.tensor_scalar_add(sgn[:], sgn[:], 0.5)
            nc.scalar.activation(sgn[:], sgn[:], AF.Sign)
            nc.scalar.activation(ab[:], pp_sb[:], AF.Abs)
            nc.vector.tensor_scalar(ab[:], ab[:], float(P_MIN), float(P_MAX),
                                    ALU.max, ALU.min)
            nc.vector.tensor_tensor(p_sb[:], sgn[:], ab[:], ALU.mult)
            nc.vector.reciprocal(ip_sb[:], p_sb[:])

            negzmax = cp.tile([P, 2], F32)
            zmin_sb = cp.tile([P, 2], F32)
            pmax = cp.tile([P, 2, NQ], F32)   # per-quarter -max partials
            pmin = cp.tile([P, 2, NQ], F32)
            # per-head [64,1] base-0 views (general-p path)
            allh = cp.tile([64, 6, 4], F32)
            iph = allh[:, 0, :]
            zmh = allh[:, 1, :]
            cmh = allh[:, 2, :]
            eih = allh[:, 5, :]
            ecb = cp.tile([P, 2, 2], BF16)  # ecp5 (or -SHIFT) hi/lo for oc
            ecl = cp.tile([P, 2], F32)

            # ---------------- emission units ----------------------------------
            def qkv_group(wsb, kind, m, nt):
                ps = psBig.tile([P, 512], F32, tag="big", name="ev")
                for kc in range(KC):
                    nc.tensor.matmul(
                        ps[:],
                        wsb[:, kc, m * P:(m + 1) * P],
                        xt_sb[:, nt, kc, :],
                        start=(kc == 0), stop=(kc == KC - 1),
                    )
                tsl = slice(nt * 512, (nt + 1) * 512)
                if kind == "q":
                    nc.vector.tensor_scalar_add(
                        qT[:, m, tsl], ps[:], bq_sb[:, m:m + 1])
                elif kind == "k":
                    nc.vector.tensor_scalar_add(
                        kT[:, m, tsl], ps[:], bk_sb[:, m:m + 1])
                    if m == 0:
                        for h in (0, 1):
                            base = 64 * (h % 2)
                            nc.scalar.dma_start(kTp[base:base + 64, h, tsl],
                                                kT[base:base + 64, m, tsl])
                else:
                    # fast (p=1): vT = |v + b + SHIFT| directly; the power-mean
                    # collapses to a plain max-normalized mean so no ln/exp is
                    # needed (keeps ScalarE on one activation table set).
                    # general: z = p * ln(|v + b + SHIFT|)
                    nc.scalar.activation(
                        vT[:, m, tsl], ps[:], AF.Abs,
                        bias=bv5_sb[:, m:m + 1])
                    if m == 0 and nt < NQ - 1:
                        nc.scalar.dma_start(xt_sb[:, nt + 1, 0:4],
                                            xt_d[:, nt + 1, 0:4])
                        nc.scalar.dma_start(xt_sb[:, nt + 1, 4:8],
                                            xt_d[:, nt + 1, 4:8])
                    elif m == 0:
                        nc.scalar.dma_start(wp_sb[:], wp_d[:])
                    if not fast_p1:
                        nc.scalar.activation(vT[:, m, tsl], vT[:, m, tsl],
                                             AF.Ln)
                        nc.vector.tensor_scalar_mul(
                            vT[:, m, tsl], vT[:, m, tsl], p_sb[:, m:m + 1])
                    nc.vector.tensor_reduce(pmax[:, m, nt:nt + 1],
                                            vT[:, m, tsl], AX.X,
                                            op=ALU.max, negate=True)
                    nc.vector.tensor_reduce(pmin[:, m, nt:nt + 1],
                                            vT[:, m, tsl], AX.X, op=ALU.min)

            def v_tail(m):
                nc.vector.tensor_reduce(negzmax[:, m:m + 1], pmax[:, m, :],
                                        AX.X, op=ALU.min)
                nc.vector.tensor_reduce(zmin_sb[:, m:m + 1], pmin[:, m, :],
                                        AX.X, op=ALU.min)
                if fast_p1:
                    # v''*ezp = |v+5| - 0.5*(max+min); ecp5 = 0.5*(max+min)-5
                    # co = 0.5*(max+min) stored in ecp5 slot (pre -SHIFT)
                    nc.vector.scalar_tensor_tensor(
                        ecp5[:, m:m + 1], negzmax[:, m:m + 1], -1.0,
                        zmin_sb[:, m:m + 1], ALU.mult, ALU.add)
                    nc.vector.tensor_scalar_mul(ecp5[:, m:m + 1],
                                                ecp5[:, m:m + 1], 0.5)
                    nc.vector.tensor_scalar_sub(vpT[:, m, :], vT[:, m, :],
                                                ecp5[:, m:m + 1])
                    nc.vector.tensor_scalar_add(ecp5[:, m:m + 1],
                                                ecp5[:, m:m + 1], -SHIFT)
                    return
                # general p: zmax/zmin; cmid = 0.5*(1+exp(zmin-zmax));
                # ezp = exp(zmax); ecp5 = ezp*cmid - 5; v'' = (v'-cmid)*ezp
                nc.vector.scalar_tensor_tensor(
                    zmaxp[:, m:m + 1], negzmax[:, m:m + 1], -1.0,
                    ip_sb[:, m:m + 1], ALU.mult, ALU.mult)
                nc.scalar.activation(cmid[:, m:m + 1], zmin_sb[:, m:m + 1],
                                     AF.Exp, bias=negzmax[:, m:m + 1])
                nc.vector.tensor_scalar(cmid[:, m:m + 1], cmid[:, m:m + 1],
                                        1.0, 0.5, ALU.add, ALU.mult)
                nc.scalar.activation(ezp[:, m:m + 1], negzmax[:, m:m + 1],
                                     AF.Exp, scale=-1.0)
                nc.scalar.activation(eip[:, m:m + 1], negzmax[:, m:m + 1],
                                     AF.Exp)
                nc.vector.scalar_tensor_tensor(
                    ecp5[:, m:m + 1], ezp[:, m:m + 1], 0.0,
                    cmid[:, m:m + 1], ALU.bypass, ALU.mult)
                nc.vector.tensor_scalar_add(ecp5[:, m:m + 1], ecp5[:, m:m + 1],
                                            -SHIFT)
                nc.scalar.activation(vT[:, m, :], vT[:, m, :], AF.Exp,
                                     bias=negzmax[:, m:m + 1])
                nc.vector.tensor_scalar(vpT[:, m, :], vT[:, m, :],
                                        cmid[:, m:m + 1], ezp[:, m:m + 1],
                                        ALU.subtract, ALU.mult)

            def kTp_fill(m):
                if m == 0:
                    return   # filled per-quarter from the k bias-adds
                for h in (2 * m, 2 * m + 1):
                    base = 64 * (h % 2)
                    nc.gpsimd.dma_start(kTp[base:base + 64, h, :],
                                        kT[base:base + 64, m, :])

            def transpose_unit(m, tp):
                # 4 key tiles -> vnat via one [128,128] PE transpose each
                trp = psX.tile([P, 512], BF16, tag="x", name="trp")
                for j in range(4):
                    kt = 4 * tp + j
                    nc.tensor.transpose(
                        trp[:, j * 128:(j + 1) * 128],
                        vpT[:, m, kt * P:(kt + 1) * P],
                        ident,
                    )
                t3 = trp[:].rearrange("p (a b) -> p a b", a=4)
                nc.vector.tensor_copy(
                    vnat[:, 2 * m, 4 * tp:4 * tp + 4, 0:64], t3[:, :, 0:64])
                nc.vector.tensor_copy(
                    vnat[:, 2 * m + 1, 4 * tp:4 * tp + 4, 0:64],
                    t3[:, :, 64:128])

            def head_consts(m):
                nc.sync.dma_start(allh[:, :, 2 * m], allcp[0:64, :, m])
                nc.sync.dma_start(allh[:, :, 2 * m + 1], allcp[64:128, :, m])

            def m_units(m):
                # main: feed-rate-matched (v,q,k per query quarter) + kTp;
                # tail: v-transform + transposes (deferred into attention
                # fillers so scores can start as soon as kTp is ready)
                units = []
                for nt in range(NQ):
                    units.append(lambda m=m, nt=nt: qkv_group(wv_sb, "v", m, nt))
                    units.append(lambda m=m, nt=nt: qkv_group(wq_sb, "q", m, nt))
                    units.append(lambda m=m, nt=nt: qkv_group(wk_sb, "k", m, nt))
                units.append(lambda m=m: kTp_fill(m))
                tail = [lambda m=m: v_tail(m)]
                for tp in range(4):
                    tail.append(lambda m=m, tp=tp: transpose_unit(m, tp))
                tail.append(lambda m=m: head_consts(m))
                return units, tail

            # ---------------- attention ---------------------------------------
            def post_pair(pv_t, hp, qi):
                # yh = num'' / den per head; const term folded into oc.
                # The very last pair splits the den chain per head so the
                # first broadcast matmul issues ~2us earlier (shorter tail,
                # PE stays HAM-warm).
                last = (hp == 1 and qi == 3)
                dcp = smp.tile([1, 1024], F32, tag="dcp", name="dcp")
                rdf = smp.tile([1, 1024], F32, tag="rdf", name="rdf")
                rd = smp.tile([1, 1024], F32R, tag="rd", name="rd")
                halves = ([(0, 512), (512, 1024)] if last else [(0, 1024)])
                for lo, hi in halves:
                    nc.vector.tensor_copy(dcp[:, lo:hi],
                                          pv_t[64:65, :, :].rearrange(
                                              "p a b -> p (a b)")[:, lo:hi])
                    nc.vector.reciprocal_approx_fast(rdf[:, lo:hi],
                                                     dcp[:, lo:hi])
                    nc.vector.tensor_copy(rd[:, lo:hi], rdf[:, lo:hi])
                me_s = smp.tile([64, 2, 512], F32, tag="me_s",
                                name="me_s")
                nc.vector.tensor_copy(me_s[:], pv_t[0:64, :, :])
                yh = smp.tile([64, 2, 512], BF16, tag="yh", name="yh")
                for i in range(2):
                    h = 2 * hp + i
                    bc = psX.tile([64, 512], F32, tag="x", name="bc",
                                  padded_shape=[P, 512])
                    nc.tensor.matmul(bc[:], onesr[:],
                                     rd[:, i * 512:(i + 1) * 512],
                                     start=True, stop=True)
                    if fast_p1:
                        nc.vector.tensor_tensor(yh[:, i, :], me_s[:, i, :],
                                                bc[:], ALU.mult)
                    else:
                        me = smp.tile([64, 512], F32, tag="me", name="me")
                        nc.vector.tensor_tensor(me[:], me_s[:, i, :], bc[:],
                                                ALU.mult)
                        nc.vector.tensor_scalar(me[:], me[:],
                                                eih[:, h:h + 1],
                                                cmh[:, h:h + 1],
                                                ALU.mult, ALU.add)
                        nc.scalar.activation(me[:], me[:], AF.Ln)
                        nc.scalar.activation(yh[:, i, :], me[:], AF.Exp,
                                             scale=iph[:, h:h + 1],
                                             bias=zmh[:, h:h + 1])
                if dbg and hp == 0 and qi == 0:
                    dpv = cp.tile([65, 1024], F32, name="dpv")
                    nc.vector.tensor_copy(
                        dpv[:], pv_t[:].rearrange("p a b -> p (a b)"))
                    dpv_d = nc.dram_tensor("dpv", [65, 1024], F32,
                                           kind="ExternalOutput")
                    nc.sync.dma_start(dpv_d[:], dpv[:])
                    dyh_d = nc.dram_tensor("dyh", [64, 1024], BF16,
                                           kind="ExternalOutput")
                    nc.sync.dma_start(dyh_d[:],
                                      yh[:].rearrange("p a b -> p (a b)"))
                    drd_d = nc.dram_tensor("drd", [2, 1024], F32,
                                           kind="ExternalOutput")
                    nc.sync.dma_start(drd_d[0:1, :], rdf[:])
                    nc.sync.dma_start(drd_d[1:2, :], dcp[:])
                qi_ = post_qi[0]
                qsl = slice(qi_ * 512, (qi_ + 1) * 512)
                yeng = nc.scalar if (hp == 1 and qi >= 2) else nc.gpsimd
                yeng.dma_start(yT[0:64, hp, qsl], yh[:, 0, :])
                yeng.dma_start(yT[64:128, hp, qsl], yh[:, 1, :])

            post_qi = [0]

            def proj_group(tq):
                po = opp.tile([P, C], BF16, tag="po", name="po")
                for nh in range(2):
                    pj = psX.tile([P, 512], F32, tag="x", name="pj")
                    for c in range(2):
                        nc.tensor.matmul(
                            pj[:],
                            yT[:, c, tq * P:(tq + 1) * P],
                            wp_sb[:, c, nh * 512:(nh + 1) * 512],
                            start=(c == 0), stop=(c == 1),
                        )
                    nc.vector.tensor_copy(po[:, nh * 512:(nh + 1) * 512],
                                          pj[:])
                    eng = nc.gpsimd if nh == 0 else (
                        nc.scalar if tq >= 12 else nc.sync)
                    eng.dma_start(
                        out_d[tq * P:(tq + 1) * P, nh * 512:(nh + 1) * 512],
                        po[:, nh * 512:(nh + 1) * 512])

            def oc_unit():
                # oc = const^T @ w_proj, const = ecp5 (fast) or -SHIFT
                # (general), hi/lo split across two bf16 stationary columns
                if fast_p1:
                    nc.vector.tensor_copy(ecb[:, :, 0], ecp5[:])
                    nc.vector.tensor_tensor(ecl[:], ecp5[:], ecb[:, :, 0],
                                            ALU.subtract)
                    nc.vector.tensor_copy(ecb[:, :, 1], ecl[:])
                else:
                    nc.vector.memset(ecb[:], 0.0)
                    nc.vector.memset(ecb[:, :, 0], -SHIFT)
                occ = cp.tile([2, C], F32)
                for nh in range(2):
                    ocp = psX.tile([2, 512], F32, tag="x", name="ocp",
                                   padded_shape=[P, 512])
                    for c in range(2):
                        nc.tensor.matmul(
                            ocp[:], ecb[:, c, :],
                            wp_sb[:, c, nh * 512:(nh + 1) * 512],
                            start=(c == 0), stop=(c == 1),
                        )
                    nc.vector.tensor_copy(occ[:, nh * 512:(nh + 1) * 512],
                                          ocp[:])
                nc.gpsimd.dma_start(oc_d[:], occ[:])

            def attention(hp, fillers):
                pending = []
                fill_i = [0]
                n_iters = sum(4 * qi + 4 for qi in range(NQ))
                it = [0]

                def pace():
                    want = min(len(fillers),
                               3 + it[0] * max(len(fillers) - 3, 0) // n_iters)
                    while fill_i[0] < want:
                        fillers[fill_i[0]]()
                        fill_i[0] += 1

                for qi in range(NQ):
                    nkt = 4 * qi + 4
                    pv = psPv.tile([65, 2, 512], F32, tag="pv", name="pv")
                    prev = None
                    for kt in range(nkt):
                        it[0] += 1
                        off = P * max(kt - 4 * qi, 0)
                        s = psBig.tile([P, 2, 512], F32, tag="big", name="s")
                        for i in range(2):
                            h = 2 * hp + i
                            nc.tensor.matmul(
                                s[:, i, off:512],
                                kTp[:, h, kt * P:(kt + 1) * P],
                                qT[:, hp, qi * 512 + off:(qi + 1) * 512],
                                start=True, stop=True,
                                skip_group_check=True,
                            )
                        pt = ptp.tile([P, 2, 512], BF16, tag="pt", name="pt")
                        nc.scalar.activation(pt[:, :, off:512],
                                             s[:, :, off:512],
                                             AF.Exp, scale=SM_SCALE)
                        if kt >= 4 * qi:   # diagonal: mask the 128-block
                            nc.vector.tensor_mul(pt[:, :, off:off + P],
                                                 pt[:, :, off:off + P],
                                                 tri2[:])
                        if dbg and hp == 0 and qi == 0 and kt == 0:
                            dpt_d = nc.dram_tensor("dpt", [P, 1024], BF16,
                                                   kind="ExternalOutput")
                            nc.sync.dma_start(
                                dpt_d[:], pt[:].rearrange("p a b -> p (a b)"))
                            ds_t = cp.tile([P, 1024], F32, name="ds_t")
                            nc.vector.tensor_copy(
                                ds_t[:], s[:].rearrange("p a b -> p (a b)"))
                            ds_d = nc.dram_tensor("ds", [P, 1024], F32,
                                                  kind="ExternalOutput")
                            nc.sync.dma_start(ds_d[:], ds_t[:])
                        pace()
                        if kt == 1:
                            for fn in pending:
                                fn()
                            pending.clear()
                            if hp == 1 and qi > 0:
                                for tq in range(4 * (qi - 1), 4 * qi):
                                    proj_group(tq)
                        if prev is not None:
                            pkt, ppt = prev
                            o_ = P * max(pkt - 4 * qi, 0)
                            for i in range(2):
                                h = 2 * hp + i
                                nc.tensor.matmul(
                                    pv[:, i, o_:512],
                                    vnat[:, h, pkt, :],
                                    ppt[:, i, o_:512],
                                    start=(pkt == 0), stop=(pkt == nkt - 1),
                                    skip_group_check=True,
                                )
                        prev = (kt, pt)
                    pkt, ppt = prev
                    o_ = P * max(pkt - 4 * qi, 0)
                    for i in range(2):
                        h = 2 * hp + i
                        nc.tensor.matmul(
                            pv[:, i, o_:512],
                            vnat[:, h, pkt, :],
                            ppt[:, i, o_:512],
                            start=(pkt == 0), stop=(pkt == nkt - 1),
                            skip_group_check=True,
                        )

                    def mk(pv_t, hp_, qi_):
                        def fn():
                            post_qi[0] = qi_
                            post_pair(pv_t, hp_, qi_)
                        return fn
                    pending.append(mk(pv, hp, qi))
                while fill_i[0] < len(fillers):
                    fillers[fill_i[0]]()
                    fill_i[0] += 1
                return pending

            # ---------------- schedule ----------------------------------------
            m0_main, m0_tail = m_units(0)
            m1_main, m1_tail = m_units(1)
            for u in m0_main:
                u()
            # interleave m0 tail (v'' + transposes) with early m1 units so
            # PE never stalls on the v'' DVE chain before head-pair-0 starts
            fill0 = [m0_tail[0], m1_main[0], m0_tail[1], m1_main[1],
                     m0_tail[2], m1_main[2], m0_tail[3], m1_main[3],
                     m0_tail[4], m0_tail[5]] + m1_main[4:] + m1_tail
            pending = attention(0, fill0)
            for fn in pending:
                fn()
            pending = attention(1, [oc_unit])
            for fn in pending:
                fn()
            for tq in range(12, 16):
                proj_group(tq)

            if dbg:
                for nm, t in [("dq", qT), ("dk", kT), ("dkp", kTp),
                              ("dvp", vpT), ("dy", yT)]:
                    dd = nc.dram_tensor(nm, list(t.shape), BF16,
                                        kind="ExternalOutput")
                    nc.sync.dma_start(dd[:], t[:])
                dvn = nc.dram_tensor("dvn", list(vnat.shape), BF16,
                                     kind="ExternalOutput")
                nc.sync.dma_start(dvn[:], vnat[:])

    nc.finalize()
    return nc


def _host_inputs(x, w_attn, b_attn, w_proj, p_param):
    """Build the 8 per-core input dicts."""
    bf16 = ml_dtypes.bfloat16
    ident = np.eye(P, dtype=np.float32)
    xx = np.arange(P, dtype=np.int64)[:, None]
    yy = np.arange(P, dtype=np.int64)[None, :]
    tri = (yy - xx >= 0).astype(np.float32)
    cst = np.concatenate([ident, tri, tri], axis=1).astype(bf16)
    onesr = np.ones((1, 64), dtype=np.float32)

    def warr(w):  # [C, n] -> [P, KC, n] contiguous
        n = w.shape[1]
        return np.ascontiguousarray(
            w.reshape(KC, P, n).transpose(1, 0, 2)).astype(bf16)

    # x^T quarters: [P, NQ, KC, 512]
    xts = []
    for b in range(B):
        xt = x[b].T.reshape(KC, P, NQ, 512)
        xts.append(np.ascontiguousarray(xt.transpose(1, 2, 0, 3)).astype(bf16))
    in_maps = []
    for core in range(8):
        b, hg = divmod(core, 4)
        cs = slice(hg * CL, (hg + 1) * CL)
        csC = slice(C + hg * CL, C + (hg + 1) * CL)
        cs2C = slice(2 * C + hg * CL, 2 * C + (hg + 1) * CL)
        in_maps.append({
            "xt": xts[b],
            "wq": warr(w_attn[:, cs]),
            "wk": warr(w_attn[:, csC]),
            "wv": warr(w_attn[:, cs2C]),
            "wp": np.ascontiguousarray(
                w_proj[cs, :].reshape(2, P, C).transpose(1, 0, 2)).astype(bf16),
            "bps": np.ascontiguousarray(np.concatenate([
                b_attn[cs].reshape(2, P).T,
                b_attn[csC].reshape(2, P).T,
                (b_attn[cs2C] + SHIFT).reshape(2, P).T,
                p_param[cs].reshape(2, P).T,
            ], axis=1).astype(np.float32)),
            "cst": cst,
            "onesr": onesr,
        })
    return in_maps


def kernel(x, w_attn, b_attn, w_proj, b_proj, p_param, _trace=False):
    x = np.asarray(x, dtype=np.float32)
    w_attn = np.asarray(w_attn, dtype=np.float32)
    b_attn = np.asarray(b_attn, dtype=np.float32)
    w_proj = np.asarray(w_proj, dtype=np.float32)
    b_proj = np.asarray(b_proj, dtype=np.float32)
    p_param = np.asarray(p_param, dtype=np.float32)

    # p == 1 admits a cheaper final transform (no per-tile ln/exp)
    p_eff = np.sign(np.sign(p_param) + 0.5) * np.clip(np.abs(p_param),
                                                      P_MIN, P_MAX)
    fast_p1 = bool(np.all(p_eff == 1.0))

    key = ("nc", fast_p1)
    if key not in _CACHE:
        _CACHE[key] = _build(fast_p1)
    nc = _CACHE[key]

    in_maps = _host_inputs(x, w_attn, b_attn, w_proj, p_param)
    res = run_bass_kernel_spmd(nc, in_maps, core_ids=list(range(8)),
                               trace=_trace)
    _CACHE["last_result"] = res

    out = np.zeros((B, T, C), dtype=np.float32)
    for core in range(8):
        b = core // 4
        out[b] += res.results[core]["out_p"].astype(np.float32)
        out[b] += res.results[core]["oc"].astype(np.float32).sum(0)
    out += b_proj[None, None, :]
    return out


if __name__ == "__main__":
    rng = np.random.default_rng(0)
    ins = {
        "x": rng.standard_normal((B, T, C), dtype=np.float32),
        "w_attn": (rng.standard_normal((C, 3 * C), dtype=np.float32) * 0.02),
        "b_attn": np.zeros(3 * C, np.float32),
        "w_proj": (rng.standard_normal((C, C), dtype=np.float32) * 0.02),
        "b_proj": np.zeros(C, np.float32),
        "p_param": np.ones(C, np.float32),
    }
    out = kernel(**ins)
    print("ran, out shape", out.shape, "finite:", np.isfinite(out).all())


# revision 32
# speedup vs baseline: 1.1352x; 1.0507x over previous
"""Trainium2 Bass kernel for nn_CausalGemAttention.

Reference computation (B=2, T=2048, C=1024, H=16, d=64):
    qkv = x @ w_attn + b_attn ; q,k,v = split(qkv)
    p = sign(sign(p_param)+0.5) * clamp(|p_param|, 1e-4, 1e3)
    vc = clip(|v + 5|, 1e-10); z = p*ln(vc); zmax = max_T(z); v' = exp(z - zmax)
    att = causal_softmax(q k^T / sqrt(d)); mean = att @ v'
    y = exp((zmax + ln(mean)) / p) - 5 ; out = y @ w_proj + b_proj

Sharding: 8 cores = 2 (batch) x 4 (head groups of 4 heads / 256 channels).
Each core computes qkv for its head group (contraction over full C), local
attention, and a partial projection (w_proj rows of its channels); host sums
the 4 partials per batch and adds b_proj.

v2 structure (single pool scope, cross-phase PSUM ring sharing):
  - PSUM map (8 banks exact): tag 'big' 2x[P,2,512]f32 (qkv accum + scores,
    double-buffered so PE never waits on softmax exp), tag 'pv' 1x[65,2,512]
    (paired PV accum), tag 'x' 2x[P,512] (proj/bc/transposes/oc).
  - attention iterates per KEY TILE with both heads of the pair in one
    scores tile; softmax exp is one strided [P,2,512-off] ACT op; only the
    128-wide diagonal block needs a triangular mask ([P,2,128] DVE mult).
  - v'' is pre-scaled by exp(zmax) per channel, and the constant term
    (exp(zmax)*cmid - 5) is folded into a per-core [2,C] vector
    oc = const^T @ w_proj (hi/lo bf16 split) added on host. The pair
    post-chain is: den copy (partition 64->0), fast reciprocal, F32R
    rounding copy, me evacuation, two K=1 broadcast matmuls + multiplies.
  - scores moving operand is raw qT (both heads packed); only the stationary
    k is zero-padded to K=128 (kTp), so no padded q buffer is needed.
  - m=1 qkv/transform units are emitted interleaved into head-pair-0
    attention (between exp and PV) to fill PE gaps; projection and
    post-chains trail one q-block behind.
  - output partials are bf16 (halves the DMA-out + PSUM evacuation cost).
"""

import sys
sys.path.insert(0, "/opt/trn_rl_repo")

import numpy as np
import ml_dtypes

import concourse.bacc as bacc
import concourse.tile as tile
from concourse import mybir
from concourse.bass_utils import run_bass_kernel_spmd

F32 = mybir.dt.float32
F32R = mybir.dt.float32r
BF16 = mybir.dt.bfloat16
AF = mybir.ActivationFunctionType
ALU = mybir.AluOpType
AX = mybir.AxisListType

B, T, C, H, D = 2, 2048, 1024, 16, 64
P = 128
CL = 256            # channels per core (4 heads x 64)
KC = C // P         # 8 contraction chunks for qkv
NQ = T // 512       # 4 query blocks of 512
NK = T // P         # 16 key tiles of 128
SHIFT = 5.0
P_MIN, P_MAX, V_MIN = 1e-4, 1e3, 1e-10
SM_SCALE = 1.0 / 8.0  # 1/sqrt(64)

# cst layout (bf16): [ident128 | tri x2]
CST_W = 128 + 256

_CACHE = {}


def _build(fast_p1, dbg=False):
    nc = bacc.Bacc("TRN2", target_bir_lowering=False, debug=False)

    xt_d = nc.dram_tensor("xt", [P, NQ, KC, 512], BF16,
                          kind="ExternalInput")
    wq_d = nc.dram_tensor("wq", [P, KC, CL], BF16, kind="ExternalInput")
    wk_d = nc.dram_tensor("wk", [P, KC, CL], BF16, kind="ExternalInput")
    wv_d = nc.dram_tensor("wv", [P, KC, CL], BF16, kind="ExternalInput")
    wp_d = nc.dram_tensor("wp", [P, 2, C], BF16, kind="ExternalInput")
    # bps = [bq(2) | bk(2) | bv5(2) | pp(2)] per chunk, fp32
    bps_d = nc.dram_tensor("bps", [P, 8], F32, kind="ExternalInput")
    cst_d = nc.dram_tensor("cst", [P, CST_W], BF16, kind="ExternalInput")
    or_d = nc.dram_tensor("onesr", [1, 64], F32R, kind="ExternalInput")
    out_d = nc.dram_tensor("out_p", [T, C], BF16, kind="ExternalOutput")
    oc_d = nc.dram_tensor("oc", [2, C], F32, kind="ExternalOutput")

    with tile.TileContext(nc) as tc:
        with (
            tc.tile_pool(name="consts", bufs=1) as cp,
            tc.tile_pool(name="main", bufs=1) as mp,
            tc.tile_pool(name="ptp", bufs=8) as ptp,
            tc.tile_pool(name="sm", bufs=2) as smp,
            tc.tile_pool(name="outp", bufs=3) as opp,
            tc.tile_pool(name="psBig", bufs=2, space="PSUM") as psBig,
            tc.tile_pool(name="psPv", bufs=1, space="PSUM") as psPv,
            tc.tile_pool(name="psX", bufs=2, space="PSUM") as psX,
        ):
            # ---------------- constants + inputs ------------------------------
            cst = cp.tile([P, CST_W], BF16)
            onesr = cp.tile([1, 64], F32R)
            bps = cp.tile([P, 8], F32)
            nc.scalar.dma_start(bps[:], bps_d[:])
            nc.scalar.dma_start(cst[:], cst_d[:])
            nc.scalar.dma_start(onesr[:], or_d[:])
            ident = cst[:, 0:128]
            tri2 = cst[:, 128:384].rearrange("p (b c) -> p b c", b=2)
            bq_sb = bps[:, 0:2]
            bk_sb = bps[:, 2:4]
            bv5_sb = bps[:, 4:6]
            pp_sb = bps[:, 6:8]

            wq_sb = mp.tile([P, KC, CL], BF16)
            wk_sb = mp.tile([P, KC, CL], BF16)
            wv_sb = mp.tile([P, KC, CL], BF16)
            wp_sb = mp.tile([P, 2, C], BF16)
            xt_sb = mp.tile([P, NQ, KC, 512], BF16)
            # all inputs are host-prearranged contiguous; x in query-block
            # quarters so the first v matmul group starts ~7us in
            # wave 1: only what the first query quarter needs, one piece
            # per trigger queue; later quarters are triggered from the
            # SCALAR queue between the per-quarter |v| ops, so they enter
            # the (round-robin) DMA queue only as the previous quarter is
            # being consumed and don't dilute wave-1 bandwidth.
            nc.scalar.dma_start(wv_sb[:], wv_d[:])
            nc.gpsimd.dma_start(wq_sb[:], wq_d[:])
            nc.gpsimd.dma_start(wk_sb[:], wk_d[:])

            qT = mp.tile([P, 2, T], BF16)    # q^T: [c%128, c//128, t]
            kT = mp.tile([P, 2, T], BF16)
            # padded per-head k (stationary side only): head h occupies
            # partitions 64*(h%2):64*(h%2)+64, rest zero
            kTp = mp.tile([P, 4, T], BF16)
            vT = mp.tile([P, 2, T], F32)
            vpT = mp.tile([P, 2, T], BF16)
            vnat = mp.tile([P, 4, NK, 65], BF16)  # [tk%128, head, tk//128, d|1]
            yT = mp.tile([P, 2, T], BF16)
            nc.gpsimd.memset(kTp[:], 0.0)
            for h in range(4):
                nc.vector.memset(vnat[:, h, :, 64], 1.0)

            # p = sign(sign(pp)+0.5) * clamp(|pp|, P_MIN, P_MAX)
            sgn = cp.tile([P, 2], F32)
            ab = cp.tile([P, 2], F32)
            p_sb = cp.tile([P, 2], F32)
            # allcp packs [ip | zmaxp | cmid | ezp | ecp5 | eip] x 2 chunks
            allcp = cp.tile([P, 6, 2], F32)
            ip_sb = allcp[:, 0, :]
            zmaxp = allcp[:, 1, :]
            cmid = allcp[:, 2, :]
            ezp = allcp[:, 3, :]
            ecp5 = allcp[:, 4, :]
            eip = allcp[:, 5, :]
            nc.scalar.activation(sgn[:], pp_sb[:], AF.Sign)
            # xt quarter 0 enters the DMA queue only after bps landed, giving
            # wv a head start on the shared round-robin DMA bandwidth
            nc.scalar.dma_start(xt_sb[:, 0, 0:4], xt_d[:, 0, 0:4])
            nc.scalar.dma_start(xt_sb[:, 0, 4:8], xt_d[:, 0, 4:8])
            nc.vector.tensor_scalar_add(sgn[:], sgn[:], 0.5)
            nc.scalar.activation(sgn[:], sgn[:], AF.Sign)
            nc.scalar.activation(ab[:], pp_sb[:], AF.Abs)
            nc.vector.tensor_scalar(ab[:], ab[:], float(P_MIN), float(P_MAX),
                                    ALU.max, ALU.min)
            nc.vector.tensor_tensor(p_sb[:], sgn[:], ab[:], ALU.mult)
            nc.vector.reciprocal(ip_sb[:], p_sb[:])

            negzmax = cp.tile([P, 2], F32)
            zmin_sb = cp.tile([P, 2], F32)
            pmax = cp.tile([P, 2, NQ], F32)   # per-quarter -max partials
            pmin = cp.tile([P, 2, NQ], F32)
            # per-head [64,1] base-0 views (general-p path)
            allh = cp.tile([64, 6, 4], F32)
            iph = allh[:, 0, :]
            zmh = allh[:, 1, :]
            cmh = allh[:, 2, :]
            eih = allh[:, 5, :]
            ecb = cp.tile([P, 2, 2], BF16)  # ecp5 (or -SHIFT) hi/lo for oc
            ecl = cp.tile([P, 2], F32)

            # ---------------- emission units ----------------------------------
            def qkv_group(wsb, kind, m, nt):
                ps = psBig.tile([P, 512], F32, tag="big", name="ev")
                for kc in range(KC):
                    nc.tensor.matmul(
                        ps[:],
                        wsb[:, kc, m * P:(m + 1) * P],
                        xt_sb[:, nt, kc, :],
                        start=(kc == 0), stop=(kc == KC - 1),
                    )
                tsl = slice(nt * 512, (nt + 1) * 512)
                if kind == "q":
                    nc.vector.tensor_scalar_add(
                        qT[:, m, tsl], ps[:], bq_sb[:, m:m + 1])
                elif kind == "k":
                    nc.vector.tensor_scalar_add(
                        kT[:, m, tsl], ps[:], bk_sb[:, m:m + 1])
                    if m == 0:
                        for h in (0, 1):
                            base = 64 * (h % 2)
                            nc.scalar.dma_start(kTp[base:base + 64, h, tsl],
                                                kT[base:base + 64, m, tsl])
                else:
                    # fast (p=1): vT = |v + b + SHIFT| directly; the power-mean
                    # collapses to a plain max-normalized mean so no ln/exp is
                    # needed (keeps ScalarE on one activation table set).
                    # general: z = p * ln(|v + b + SHIFT|)
                    nc.scalar.activation(
                        vT[:, m, tsl], ps[:], AF.Abs,
                        bias=bv5_sb[:, m:m + 1])
                    if m == 0 and nt < NQ - 1:
                        nc.scalar.dma_start(xt_sb[:, nt + 1, 0:4],
                                            xt_d[:, nt + 1, 0:4])
                        nc.scalar.dma_start(xt_sb[:, nt + 1, 4:8],
                                            xt_d[:, nt + 1, 4:8])
                    elif m == 0:
                        nc.scalar.dma_start(wp_sb[:], wp_d[:])
                    if not fast_p1:
                        nc.scalar.activation(vT[:, m, tsl], vT[:, m, tsl],
                                             AF.Ln)
                        nc.vector.tensor_scalar_mul(
                            vT[:, m, tsl], vT[:, m, tsl], p_sb[:, m:m + 1])
                    nc.vector.tensor_reduce(pmax[:, m, nt:nt + 1],
                                            vT[:, m, tsl], AX.X,
                                            op=ALU.max, negate=True)
                    nc.vector.tensor_reduce(pmin[:, m, nt:nt + 1],
                                            vT[:, m, tsl], AX.X, op=ALU.min)

            def v_tail(m):
                nc.vector.tensor_reduce(negzmax[:, m:m + 1], pmax[:, m, :],
                                        AX.X, op=ALU.min)
                nc.vector.tensor_reduce(zmin_sb[:, m:m + 1], pmin[:, m, :],
                                        AX.X, op=ALU.min)
                if fast_p1:
                    # v''*ezp = |v+5| - 0.5*(max+min); ecp5 = 0.5*(max+min)-5
                    # co = 0.5*(max+min) stored in ecp5 slot (pre -SHIFT)
                    nc.vector.scalar_tensor_tensor(
                        ecp5[:, m:m + 1], negzmax[:, m:m + 1], -1.0,
                        zmin_sb[:, m:m + 1], ALU.mult, ALU.add)
                    nc.vector.tensor_scalar_mul(ecp5[:, m:m + 1],
                                                ecp5[:, m:m + 1], 0.5)
                    nc.vector.tensor_scalar_sub(vpT[:, m, :], vT[:, m, :],
                                                ecp5[:, m:m + 1])
                    nc.vector.tensor_scalar_add(ecp5[:, m:m + 1],
                                                ecp5[:, m:m + 1], -SHIFT)
                    return
                # general p: zmax/zmin; cmid = 0.5*(1+exp(zmin-zmax));
                # ezp = exp(zmax); ecp5 = ezp*cmid - 5; v'' = (v'-cmid)*ezp
                nc.vector.scalar_tensor_tensor(
                    zmaxp[:, m:m + 1], negzmax[:, m:m + 1], -1.0,
                    ip_sb[:, m:m + 1], ALU.mult, ALU.mult)
                nc.scalar.activation(cmid[:, m:m + 1], zmin_sb[:, m:m + 1],
                                     AF.Exp, bias=negzmax[:, m:m + 1])
                nc.vector.tensor_scalar(cmid[:, m:m + 1], cmid[:, m:m + 1],
                                        1.0, 0.5, ALU.add, ALU.mult)
                nc.scalar.activation(ezp[:, m:m + 1], negzmax[:, m:m + 1],
                                     AF.Exp, scale=-1.0)
                nc.scalar.activation(eip[:, m:m + 1], negzmax[:, m:m + 1],
                                     AF.Exp)
                nc.vector.scalar_tensor_tensor(
                    ecp5[:, m:m + 1], ezp[:, m:m + 1], 0.0,
                    cmid[:, m:m + 1], ALU.bypass, ALU.mult)
                nc.vector.tensor_scalar_add(ecp5[:, m:m + 1], ecp5[:, m:m + 1],
                                            -SHIFT)
                nc.scalar.activation(vT[:, m, :], vT[:, m, :], AF.Exp,
                                     bias=negzmax[:, m:m + 1])
                nc.vector.tensor_scalar(vpT[:, m, :], vT[:, m, :],
                                        cmid[:, m:m + 1], ezp[:, m:m + 1],
                                        ALU.subtract, ALU.mult)

            def kTp_fill(m):
                if m == 0:
                    return   # filled per-quarter from the k bias-adds
                for h in (2 * m, 2 * m + 1):
                    base = 64 * (h % 2)
                    nc.gpsimd.dma_start(kTp[base:base + 64, h, :],
                                        kT[base:base + 64, m, :])

            def transpose_unit(m, tp):
                # 4 key tiles -> vnat via one [128,128] PE transpose each
                trp = psX.tile([P, 512], BF16, tag="x", name="trp")
                for j in range(4):
                    kt = 4 * tp + j
                    nc.tensor.transpose(
                        trp[:, j * 128:(j + 1) * 128],
                        vpT[:, m, kt * P:(kt + 1) * P],
                        ident,
                    )
                t3 = trp[:].rearrange("p (a b) -> p a b", a=4)
                nc.vector.tensor_copy(
                    vnat[:, 2 * m, 4 * tp:4 * tp + 4, 0:64], t3[:, :, 0:64])
                nc.vector.tensor_copy(
                    vnat[:, 2 * m + 1, 4 * tp:4 * tp + 4, 0:64],
                    t3[:, :, 64:128])

            def head_consts(m):
                nc.sync.dma_start(allh[:, :, 2 * m], allcp[0:64, :, m])
                nc.sync.dma_start(allh[:, :, 2 * m + 1], allcp[64:128, :, m])

            def m_units(m):
                # main: feed-rate-matched (v,q,k per query quarter) + kTp;
                # tail: v-transform + transposes (deferred into attention
                # fillers so scores can start as soon as kTp is ready)
                units = []
                for nt in range(NQ):
                    units.append(lambda m=m, nt=nt: qkv_group(wv_sb, "v", m, nt))
                    units.append(lambda m=m, nt=nt: qkv_group(wq_sb, "q", m, nt))
                    units.append(lambda m=m, nt=nt: qkv_group(wk_sb, "k", m, nt))
                units.append(lambda m=m: kTp_fill(m))
                tail = [lambda m=m: v_tail(m)]
                for tp in range(4):
                    tail.append(lambda m=m, tp=tp: transpose_unit(m, tp))
                tail.append(lambda m=m: head_consts(m))
                return units, tail

            # ---------------- attention ---------------------------------------
            def post_pair(pv_t, hp, qi):
                # yh = num'' / den per head; const term folded into oc.
                # The very last pair splits the den chain per head so the
                # first broadcast matmul issues ~2us earlier (shorter tail,
                # PE stays HAM-warm).
                last = (hp == 1 and qi == 3)
                dcp = smp.tile([1, 1024], F32, tag="dcp", name="dcp")
                rdf = smp.tile([1, 1024], F32, tag="rdf", name="rdf")
                rd = smp.tile([1, 1024], F32R, tag="rd", name="rd")
                halves = ([(0, 512), (512, 1024)] if last else [(0, 1024)])
                for lo, hi in halves:
                    nc.vector.tensor_copy(dcp[:, lo:hi],
                                          pv_t[64:65, :, :].rearrange(
                                              "p a b -> p (a b)")[:, lo:hi])
                    nc.vector.reciprocal_approx_fast(rdf[:, lo:hi],
                                                     dcp[:, lo:hi])
                    nc.vector.tensor_copy(rd[:, lo:hi], rdf[:, lo:hi])
                me_s = smp.tile([64, 2, 512], F32, tag="me_s",
                                name="me_s")
                nc.vector.tensor_copy(me_s[:], pv_t[0:64, :, :])
                yh = smp.tile([64, 2, 512], BF16, tag="yh", name="yh")
                for i in range(2):
                    h = 2 * hp + i
                    bc = psX.tile([64, 512], F32, tag="x", name="bc",
                                  padded_shape=[P, 512])
                    nc.tensor.matmul(bc[:], onesr[:],
                                     rd[:, i * 512:(i + 1) * 512],
                                     start=True, stop=True)
                    if fast_p1:
                        nc.vector.tensor_tensor(yh[:, i, :], me_s[:, i, :],
                                                bc[:], ALU.mult)
                    else:
                        me = smp.tile([64, 512], F32, tag="me", name="me")
                        nc.vector.tensor_tensor(me[:], me_s[:, i, :], bc[:],
                                                ALU.mult)
                        nc.vector.tensor_scalar(me[:], me[:],
                                                eih[:, h:h + 1],
                                                cmh[:, h:h + 1],
                                                ALU.mult, ALU.add)
                        nc.scalar.activation(me[:], me[:], AF.Ln)
                        nc.scalar.activation(yh[:, i, :], me[:], AF.Exp,
                                             scale=iph[:, h:h + 1],
                                             bias=zmh[:, h:h + 1])
                if dbg and hp == 0 and qi == 0:
                    dpv = cp.tile([65, 1024], F32, name="dpv")
                    nc.vector.tensor_copy(
                        dpv[:], pv_t[:].rearrange("p a b -> p (a b)"))
                    dpv_d = nc.dram_tensor("dpv", [65, 1024], F32,
                                           kind="ExternalOutput")
                    nc.sync.dma_start(dpv_d[:], dpv[:])
                    dyh_d = nc.dram_tensor("dyh", [64, 1024], BF16,
                                           kind="ExternalOutput")
                    nc.sync.dma_start(dyh_d[:],
                                      yh[:].rearrange("p a b -> p (a b)"))
                    drd_d = nc.dram_tensor("drd", [2, 1024], F32,
                                           kind="ExternalOutput")
                    nc.sync.dma_start(drd_d[0:1, :], rdf[:])
                    nc.sync.dma_start(drd_d[1:2, :], dcp[:])
                qi_ = post_qi[0]
                qsl = slice(qi_ * 512, (qi_ + 1) * 512)
                yeng = nc.scalar if (hp == 1 and qi >= 2) else nc.gpsimd
                yeng.dma_start(yT[0:64, hp, qsl], yh[:, 0, :])
                yeng.dma_start(yT[64:128, hp, qsl], yh[:, 1, :])

            post_qi = [0]

            def proj_group(tq):
                po = opp.tile([P, C], BF16, tag="po", name="po")
                for nh in range(2):
                    pj = psX.tile([P, 512], F32, tag="x", name="pj")
                    for c in range(2):
                        nc.tensor.matmul(
                            pj[:],
                            yT[:, c, tq * P:(tq + 1) * P],
                            wp_sb[:, c, nh * 512:(nh + 1) * 512],
                            start=(c == 0), stop=(c == 1),
                        )
                    nc.vector.tensor_copy(po[:, nh * 512:(nh + 1) * 512],
                                          pj[:])
                    eng = nc.gpsimd if nh == 0 else (
                        nc.scalar if tq >= 12 else nc.sync)
                    eng.dma_start(
                        out_d[tq * P:(tq + 1) * P, nh * 512:(nh + 1) * 512],
                        po[:, nh * 512:(nh + 1) * 512])

            def oc_unit():
                # oc = const^T @ w_proj, const = ecp5 (fast) or -SHIFT
                # (general), hi/lo split across two bf16 stationary columns
                if fast_p1:
                    nc.vector.tensor_copy(ecb[:, :, 0], ecp5[:])
                    nc.vector.tensor_tensor(ecl[:], ecp5[:], ecb[:, :, 0],
                                            ALU.subtract)
                    nc.vector.tensor_copy(ecb[:, :, 1], ecl[:])
                else:
                    nc.vector.memset(ecb[:], 0.0)
                    nc.vector.memset(ecb[:, :, 0], -SHIFT)
                occ = cp.tile([2, C], F32)
                for nh in range(2):
                    ocp = psX.tile([2, 512], F32, tag="x", name="ocp",
                                   padded_shape=[P, 512])
                    for c in range(2):
                        nc.tensor.matmul(
                            ocp[:], ecb[:, c, :],
                            wp_sb[:, c, nh * 512:(nh + 1) * 512],
                            start=(c == 0), stop=(c == 1),
                        )
                    nc.vector.tensor_copy(occ[:, nh * 512:(nh + 1) * 512],
                                          ocp[:])
                nc.gpsimd.dma_start(oc_d[:], occ[:])

            def attention(hp, fillers):
                pending = []
                fill_i = [0]
                n_iters = sum(4 * qi + 4 for qi in range(NQ))
                it = [0]

                def pace():
                    want = min(len(fillers),
                               3 + it[0] * max(len(fillers) - 3, 0) // n_iters)
                    while fill_i[0] < want:
                        fillers[fill_i[0]]()
                        fill_i[0] += 1

                for qi in range(NQ):
                    nkt = 4 * qi + 4
                    pv = psPv.tile([65, 2, 512], F32, tag="pv", name="pv")
                    prevs = []
                    for kt in range(nkt):
                        it[0] += 1
                        off = P * max(kt - 4 * qi, 0)
                        s = psBig.tile([P, 2, 512], F32, tag="big", name="s")
                        for i in range(2):
                            h = 2 * hp + i
                            nc.tensor.matmul(
                                s[:, i, off:512],
                                kTp[:, h, kt * P:(kt + 1) * P],
                                qT[:, hp, qi * 512 + off:(qi + 1) * 512],
                                start=True, stop=True,
                                skip_group_check=True,
                            )
                        pt = ptp.tile([P, 2, 512], BF16, tag="pt", name="pt")
                        nc.scalar.activation(pt[:, :, off:512],
                                             s[:, :, off:512],
                                             AF.Exp, scale=SM_SCALE)
                        if kt >= 4 * qi:   # diagonal: mask the 128-block
                            nc.vector.tensor_mul(pt[:, :, off:off + P],
                                                 pt[:, :, off:off + P],
                                                 tri2[:])
                        if dbg and hp == 0 and qi == 0 and kt == 0:
                            dpt_d = nc.dram_tensor("dpt", [P, 1024], BF16,
                                                   kind="ExternalOutput")
                            nc.sync.dma_start(
                                dpt_d[:], pt[:].rearrange("p a b -> p (a b)"))
                            ds_t = cp.tile([P, 1024], F32, name="ds_t")
                            nc.vector.tensor_copy(
                                ds_t[:], s[:].rearrange("p a b -> p (a b)"))
                            ds_d = nc.dram_tensor("ds", [P, 1024], F32,
                                                  kind="ExternalOutput")
                            nc.sync.dma_start(ds_d[:], ds_t[:])
                        pace()
                        if kt == 1:
                            for fn in pending:
                                fn()
                            pending.clear()
                            if hp == 1 and qi > 0:
                                for tq in range(4 * (qi - 1), 4 * qi):
                                    proj_group(tq)
                        if len(prevs) >= 2:
                            pkt, ppt = prevs.pop(0)
                            o_ = P * max(pkt - 4 * qi, 0)
                            for i in range(2):
                                h = 2 * hp + i
                                nc.tensor.matmul(
                                    pv[:, i, o_:512],
                                    vnat[:, h, pkt, :],
                                    ppt[:, i, o_:512],
                                    start=(pkt == 0), stop=(pkt == nkt - 1),
                                    skip_group_check=True,
                                )
                        prevs.append((kt, pt))
                    for pkt, ppt in prevs:
                        o_ = P * max(pkt - 4 * qi, 0)
                        for i in range(2):
                            h = 2 * hp + i
                            nc.tensor.matmul(
                                pv[:, i, o_:512],
                                vnat[:, h, pkt, :],
                                ppt[:, i, o_:512],
                                start=(pkt == 0), stop=(pkt == nkt - 1),
                                skip_group_check=True,
                            )
                    prevs.clear()

                    def mk(pv_t, hp_, qi_):
                        def fn():
                            post_qi[0] = qi_
                            post_pair(pv_t, hp_, qi_)
                        return fn
                    pending.append(mk(pv, hp, qi))
                while fill_i[0] < len(fillers):
                    fillers[fill_i[0]]()
                    fill_i[0] += 1
                return pending

            # ---------------- schedule ----------------------------------------
            m0_main, m0_tail = m_units(0)
            m1_main, m1_tail = m_units(1)
            for u in m0_main:
                u()
            # interleave m0 tail (v'' + transposes) with early m1 units so
            # PE never stalls on the v'' DVE chain before head-pair-0 starts
            fill0 = [m0_tail[0], m1_main[0], m0_tail[1], m1_main[1],
                     m0_tail[2], m1_main[2], m0_tail[3], m1_main[3],
                     m0_tail[4], m0_tail[5]] + m1_main[4:] + m1_tail
            pending = attention(0, fill0)
            for fn in pending:
                fn()
            pending = attention(1, [oc_unit])
            for fn in pending:
                fn()
            for tq in range(12, 16):
                proj_group(tq)

            if dbg:
                for nm, t in [("dq", qT), ("dk", kT), ("dkp", kTp),
                              ("dvp", vpT), ("dy", yT)]:
                    dd = nc.dram_tensor(nm, list(t.shape), BF16,
                                        kind="ExternalOutput")
                    nc.sync.dma_start(dd[:], t[:])
                dvn = nc.dram_tensor("dvn", list(vnat.shape), BF16,
                                     kind="ExternalOutput")
                nc.sync.dma_start(dvn[:], vnat[:])

    nc.finalize()
    return nc


def _host_inputs(x, w_attn, b_attn, w_proj, p_param):
    """Build the 8 per-core input dicts."""
    bf16 = ml_dtypes.bfloat16
    ident = np.eye(P, dtype=np.float32)
    xx = np.arange(P, dtype=np.int64)[:, None]
    yy = np.arange(P, dtype=np.int64)[None, :]
    tri = (yy - xx >= 0).astype(np.float32)
    cst = np.concatenate([ident, tri, tri], axis=1).astype(bf16)
    onesr = np.ones((1, 64), dtype=np.float32)

    def warr(w):  # [C, n] -> [P, KC, n] contiguous
        n = w.shape[1]
        return np.ascontiguousarray(
            w.reshape(KC, P, n).transpose(1, 0, 2)).astype(bf16)

    # x^T quarters: [P, NQ, KC, 512]
    xts = []
    for b in range(B):
        xt = x[b].T.reshape(KC, P, NQ, 512)
        xts.append(np.ascontiguousarray(xt.transpose(1, 2, 0, 3)).astype(bf16))
    in_maps = []
    for core in range(8):
        b, hg = divmod(core, 4)
        cs = slice(hg * CL, (hg + 1) * CL)
        csC = slice(C + hg * CL, C + (hg + 1) * CL)
        cs2C = slice(2 * C + hg * CL, 2 * C + (hg + 1) * CL)
        in_maps.append({
            "xt": xts[b],
            "wq": warr(w_attn[:, cs]),
            "wk": warr(w_attn[:, csC]),
            "wv": warr(w_attn[:, cs2C]),
            "wp": np.ascontiguousarray(
                w_proj[cs, :].reshape(2, P, C).transpose(1, 0, 2)).astype(bf16),
            "bps": np.ascontiguousarray(np.concatenate([
                b_attn[cs].reshape(2, P).T,
                b_attn[csC].reshape(2, P).T,
                (b_attn[cs2C] + SHIFT).reshape(2, P).T,
                p_param[cs].reshape(2, P).T,
            ], axis=1).astype(np.float32)),
            "cst": cst,
            "onesr": onesr,
        })
    return in_maps


def kernel(x, w_attn, b_attn, w_proj, b_proj, p_param, _trace=False):
    x = np.asarray(x, dtype=np.float32)
    w_attn = np.asarray(w_attn, dtype=np.float32)
    b_attn = np.asarray(b_attn, dtype=np.float32)
    w_proj = np.asarray(w_proj, dtype=np.float32)
    b_proj = np.asarray(b_proj, dtype=np.float32)
    p_param = np.asarray(p_param, dtype=np.float32)

    # p == 1 admits a cheaper final transform (no per-tile ln/exp)
    p_eff = np.sign(np.sign(p_param) + 0.5) * np.clip(np.abs(p_param),
                                                      P_MIN, P_MAX)
    fast_p1 = bool(np.all(p_eff == 1.0))

    key = ("nc", fast_p1)
    if key not in _CACHE:
        _CACHE[key] = _build(fast_p1)
    nc = _CACHE[key]

    in_maps = _host_inputs(x, w_attn, b_attn, w_proj, p_param)
    res = run_bass_kernel_spmd(nc, in_maps, core_ids=list(range(8)),
                               trace=_trace)
    _CACHE["last_result"] = res

    out = np.zeros((B, T, C), dtype=np.float32)
    for core in range(8):
        b = core // 4
        out[b] += res.results[core]["out_p"].astype(np.float32)
        out[b] += res.results[core]["oc"].astype(np.float32).sum(0)
    out += b_proj[None, None, :]
    return out


if __name__ == "__main__":
    rng = np.random.default_rng(0)
    ins = {
        "x": rng.standard_normal((B, T, C), dtype=np.float32),
        "w_attn": (rng.standard_normal((C, 3 * C), dtype=np.float32) * 0.02),
        "b_attn": np.zeros(3 * C, np.float32),
        "w_proj": (rng.standard_normal((C, C), dtype=np.float32) * 0.02),
        "b_proj": np.zeros(C, np.float32),
        "p_param": np.ones(C, np.float32),
    }
    out = kernel(**ins)
    print("ran, out shape", out.shape, "finite:", np.isfinite(out).all())
